# revision 3
# baseline (speedup 1.0000x reference)
"""CHGCNN hypergraph-conv forward on 8 Trainium2 NeuronCores (Bass/Tile).

Strategy (per core, SPMD single NEFF):
  - Edges sharded across 8 cores (dealt round-robin by degree). Nodes canonical 0..N-1.
  - Layer l: T table [N,64] (h_l) in each core's HBM (replicated).
    P1: e_feat_piece[(e,src_block)] = sum of T[node] over pairs (gather chunks via
        dma_gather + per-slot scale + strided DVE window reduce + dma_scatter_add of
        unique piece rows into per-(srcblk, dsthalf) regions).
    L2-P1: e_feat = sum of 4 piece regions (plain DMA + strided reduce, regions pre-zeroed).
    P2: partial[(n, e_half)] pieces from gathers of e_feat (scale = Binv*Dinv per pair),
        scattered into per-(e_half, n_quarter) regions.
    L2-P2: ARIN[n] = sum of 2 pieces  (canonical order, sequential).
    AllReduce(ARIN) -> ARO (full raw2 = Dinv*A*Binv*A^T h, all cores).
    Dense: stats via augmented matmul C=[h|1]^T[h|1]; BN folded into W'=W*diag(a),
    c'=a*b+beta-a*mean_y; apply: h_{l+1} = softplus(raw2 @ W' + c') via PE-transpose+matmul.
  - Pooling: per-graph mean via small matmuls; FC stack; out [512,1] (identical on cores).
"""
import sys, os
sys.path.insert(0, "/opt/trn_rl_repo")
import numpy as np

class _KBDone(Exception):
    pass

LAST_EXEC_NS = None

P = 128
NCORES = 8
MAXNI = 1024          # validated dma_gather/scatter max rows per instruction
MAXCOLS = MAXNI // P  # 8 columns per chunk
BLK = 25000           # node table block size for int16 gathers (<32768)

# ----------------------------------------------------------------------------
# Host schedule construction
# ----------------------------------------------------------------------------

def _wrap16(v):
    """dma_gather/scatter idx layout: slot i -> [i%16, i//16], replicated to 128 rows."""
    n = len(v)
    assert n % 16 == 0
    w = np.asarray(v, np.int16).reshape(n // 16, 16).T
    return np.tile(w, (8, 1))  # [128, n//16]


class AggSched:
    """Uniform (cross-core) schedule for one gather->reduce->scatter pass.

    Per core data streams: idx (int16 wrapped), scale (fp32 [128, cols]),
    sidx (int16 wrapped scatter rows).
    chunks: list of (src_row_off, n_slots, idx_coloff, scale_coloff,
                     runs=[(col0, nw, D)], region, n_rows, sidx_off, serial)
    """
    def __init__(self):
        self.chunks = []
        self.idx = [[] for _ in range(NCORES)]     # per-core list of [128, k] blocks
        self.scale = [[] for _ in range(NCORES)]   # per-core list of [128, c] blocks
        self.sidx = [[] for _ in range(NCORES)]
        self.idx_cols = 0
        self.scale_cols = 0
        self.sidx_cols = 0


def build_agg(per_core_segs, src_block_of, src_off_of, n_regions, region_rows):
    """per_core_segs: dict key -> list over cores of lists of
         (dst_region, dst_row, [(src_row_global, scale), ...])
       key must include (region, src_block); all segs of a key share both.
       src_block_of/src_off_of: key -> src block row offset (for gather in_ap)
       Region dump row = region_rows[r] (scatter pad target).
    Returns AggSched. Segments are padded across cores for SPMD uniformity.
    """
    s = AggSched()
    for key in sorted(per_core_segs.keys()):
        percore = per_core_segs[key]
        region = None
        nseg = max(len(x) for x in percore)
        if nseg == 0:
            continue
        # sort by count desc per core
        percore = [sorted(x, key=lambda t: -len(t[2])) for x in percore]
        src_off = src_off_of(key)
        # windows of 128 segments
        nwin = (nseg + P - 1) // P
        w0 = 0
        pend_windows = []  # (D, per-core [(dst_row, srcs, scales)] x128)
        for w in range(nwin):
            lo = w * P
            D = 0
            for c in range(NCORES):
                if lo < len(percore[c]):
                    D = max(D, len(percore[c][lo][2]))
            D = max(D, 1)
            win = []
            for c in range(NCORES):
                rows = []
                for p in range(P):
                    i = lo + p
                    if i < len(percore[c]):
                        reg, dst, pairs = percore[c][i]
                        rows.append((dst, pairs))
                    else:
                        rows.append((None, []))
                win.append(rows)
            region = key[0]
            pend_windows.append((D, win))
        # chunk windows: sum of D <= MAXCOLS
        i = 0
        while i < len(pend_windows):
            j = i
            tot = 0
            while j < len(pend_windows) and tot + pend_windows[j][0] <= MAXCOLS:
                tot += pend_windows[j][0]
                j += 1
            group = pend_windows[i:j]
            i = j
            # emit chunk
            cols = sum(d for d, _ in group)
            nw = len(group)
            idx_block = np.zeros((NCORES, cols * P), np.int64)
            sc_block = np.zeros((NCORES, P, cols), np.float32)
            sidx_block = np.zeros((NCORES, nw * P), np.int64)
            runs = []
            col0 = 0
            rr = region_rows[key[0]]
            for wi, (D, win) in enumerate(group):
                # merge equal-D consecutive into runs
                if runs and runs[-1][2] == D and runs[-1][0] + runs[-1][1] * runs[-1][2] == col0:
                    runs[-1] = (runs[-1][0], runs[-1][1] + 1, D)
                else:
                    runs.append((col0, 1, D))
                for c in range(NCORES):
                    for p in range(P):
                        dst, pairs = win[c][p]
                        sidx_block[c, wi * P + p] = rr if dst is None else dst
                        for t in range(D):
                            col = col0 + t
                            slot = col * P + p
                            if t < len(pairs):
                                sr, sv = pairs[t]
                                idx_block[c, slot] = sr - src_off
                                sc_block[c, p, col] = sv
                            else:
                                idx_block[c, slot] = 0
                                sc_block[c, p, col] = 0.0
                col0 += D
            for c in range(NCORES):
                s.idx[c].append(_wrap16(idx_block[c]))
                s.scale[c].append(sc_block[c])
                s.sidx[c].append(_wrap16(sidx_block[c]))
            s.chunks.append(dict(
                src_off=src_off, n_slots=cols * P, cols=cols,
                idx_off=s.idx_cols, scale_off=s.scale_cols,
                runs=runs, region=region, n_rows=nw * P, nw=nw,
                sidx_off=s.sidx_cols))
            s.idx_cols += cols * P // 16
            s.scale_cols += cols
            s.sidx_cols += nw * P // 16
    return s


def host_prep(x, node_idx, edge_idx, batch):
    N = x.shape[0]
    I = node_idx.shape[0]
    E = int(edge_idx.max()) + 1 if len(edge_idx) else 1
    G = int(batch.max()) + 1
    node_idx = np.asarray(node_idx, np.int64)
    edge_idx = np.asarray(edge_idx, np.int64)
    batch = np.asarray(batch, np.int64)

    deg_e = np.bincount(edge_idx, minlength=E)
    deg_n = np.bincount(node_idx, minlength=N)
    Binv = np.where(deg_e > 0, 1.0 / np.maximum(deg_e, 1), 0.0).astype(np.float32)
    Dinv = np.where(deg_n > 0, 1.0 / np.maximum(deg_n, 1), 0.0).astype(np.float32)

    # ---- edge shard: live edges dealt by degree
    live = np.nonzero(deg_e > 0)[0]
    order = live[np.argsort(-deg_e[live], kind="stable")]
    core_of_edge = np.full(E, -1, np.int64)
    epos = np.full(E, -1, np.int64)   # local e_feat row
    counts = [0] * NCORES
    for i, e in enumerate(order):
        c = i % NCORES
        core_of_edge[e] = c
        epos[e] = counts[c]
        counts[c] += 1
    RE = max(counts)
    # e_feat halves (128-aligned so EF half offsets match stream indices)
    EH = (((RE + 1) // 2 + 127) // 128) * 128
    assert EH <= 32767

    # pairs grouped per (core)
    pair_core = core_of_edge[edge_idx]

    # ---- P1 segments: key (region=(srcblk, dsthalf)), seg=(dst=piece row, srcs)
    nblk = (N + BLK - 1) // BLK
    p1_regions = {}
    for b in range(nblk):
        for h in range(2):
            p1_regions[(b, h)] = len(p1_regions)
    p1_region_size = EH + 1  # + dump row

    segs1 = {}
    src_blk = node_idx // BLK
    for c in range(NCORES):
        sel = np.nonzero(pair_core == c)[0]
        ni = node_idx[sel]
        ei = edge_idx[sel]
        bi = src_blk[sel]
        ep = epos[ei]
        hh = (ep >= EH).astype(np.int64)
        erow = ep - hh * EH
        # group by (b, h, e)
        key_arr = (bi * 2 + hh) * (EH + 1) + erow
        o = np.argsort(key_arr, kind="stable")
        ni, bi, hh, erow = ni[o], bi[o], hh[o], erow[o]
        ka = key_arr[o]
        bounds = np.nonzero(np.diff(ka))[0] + 1
        starts = np.concatenate([[0], bounds])
        ends = np.concatenate([bounds, [len(ka)]])
        for si, se in zip(starts, ends):
            b = int(bi[si]); h = int(hh[si]); r = int(erow[si])
            key = (b, h)
            reg = p1_regions[key]
            segs1.setdefault((reg, b), [[] for _ in range(NCORES)])[c].append(
                (reg, r, [(int(n), 1.0) for n in ni[si:se]]))
    # split counts > MAXCOLS
    _split_big(segs1, MAXCOLS)
    sched1 = build_agg(segs1, None, lambda k: (k[1]) * BLK, len(p1_regions),
                       {p1_regions[k]: EH for k in p1_regions})

    # ---- L2-P1: e_feat[rows RE] = sum over nblk regions at same (h, row)
    # handled densely in kernel (no schedule needed beyond sizes)

    # ---- P2 segments: dst piece (n, e_half) -> region (h, quarter(n))
    NQ = (N + 3) // 4
    assert NQ <= 32767
    p2_regions = {}
    for h in range(2):
        for q in range(4):
            p2_regions[(h, q)] = len(p2_regions)
    p2_region_size = NQ + 1

    segs2 = {}
    sc2 = Binv[edge_idx] * Dinv[node_idx]
    for c in range(NCORES):
        sel = np.nonzero(pair_core == c)[0]
        ni = node_idx[sel]
        ei = edge_idx[sel]
        sv = sc2[sel]
        ep = epos[ei]
        hh = (ep >= EH).astype(np.int64)
        erow = ep - hh * EH
        qq = ni // NQ
        nrow = ni - qq * NQ
        key_arr = ((hh * 4 + qq) * (NQ + 1) + nrow)
        o = np.argsort(key_arr, kind="stable")
        ni, sv, hh, erow, qq, nrow = ni[o], sv[o], hh[o], erow[o], qq[o], nrow[o]
        ka = key_arr[o]
        bounds = np.nonzero(np.diff(ka))[0] + 1
        starts = np.concatenate([[0], bounds])
        ends = np.concatenate([bounds, [len(ka)]])
        for si, se in zip(starts, ends):
            h = int(hh[si]); q = int(qq[si]); r = int(nrow[si])
            reg = p2_regions[(h, q)]
            segs2.setdefault((reg, h), [[] for _ in range(NCORES)])[c].append(
                (reg, r, [(int(h * EH + er), float(s)) for er, s in
                          zip(erow[si:se], sv[si:se])]))
    _split_big(segs2, MAXCOLS)
    sched2 = build_agg(segs2, None, lambda k: (k[1]) * EH, len(p2_regions),
                       {p2_regions[k]: NQ for k in p2_regions})

    # ---- pooling: per node-tile matmul pieces
    # graphs contiguous in node order (batch sorted)
    cnt_g = np.bincount(batch, minlength=G).astype(np.float32)
    pool_meta = []   # (tile, window, col_lo_graph, ncols, startflags per window handled in build)
    NT = (N + P - 1) // P
    pool_cols = []
    for t in range(NT):
        lo, hi = t * P, min((t + 1) * P, N)
        gs = batch[lo:hi]
        g0, g1 = int(gs[0]), int(gs[-1])
        # windows of 128 graphs
        w0, w1 = g0 // P, g1 // P
        for w in range(w0, w1 + 1):
            glo = max(g0, w * P)
            ghi = min(g1, (w + 1) * P - 1)
            ncol = ghi - glo + 1
            Pt = np.zeros((P, ncol), np.float32)
            for p in range(hi - lo):
                g = int(gs[p])
                if w * P <= g <= ghi and g >= glo:
                    Pt[p, g - glo] = 1.0 / max(cnt_g[g], 1.0)
            pool_meta.append(dict(tile=t, window=w, grow=glo - w * P, ncol=ncol,
                                  col_off=sum(c.shape[1] for c in pool_cols)))
            pool_cols.append(Pt)
    pool_data = np.concatenate(pool_cols, axis=1) if pool_cols else np.zeros((P, 1), np.float32)

    return dict(
        N=N, E=E, I=I, G=G, RE=RE, EH=EH, NQ=NQ, nblk=nblk,
        deg_e=deg_e, deg_n=deg_n, Binv=Binv, Dinv=Dinv,
        core_of_edge=core_of_edge, epos=epos,
        sched1=sched1, sched2=sched2,
        n_p1_regions=len(p1_regions), p1_region_size=p1_region_size,
        n_p2_regions=len(p2_regions), p2_region_size=p2_region_size,
        pool_meta=pool_meta, pool_data=pool_data, cnt_g=cnt_g,
    )


def _split_big(segs, maxc):
    # Oversized segments are split into levels; each level becomes its OWN key so
    # duplicate destinations never share a scatter instruction (CCE RMW races).
    for key in list(segs):
        base = segs.pop(key)
        levels = {}
        for c in range(NCORES):
            for reg, dst, pairs in base[c]:
                for lv, j in enumerate(range(0, len(pairs), maxc)):
                    levels.setdefault(lv, [[] for _ in range(NCORES)])[c].append(
                        (reg, dst, pairs[j:j + maxc]))
        for lv, percore in levels.items():
            segs[key + (lv,)] = percore


# ----------------------------------------------------------------------------
# numpy executor for schedule validation (mirrors device semantics)
# ----------------------------------------------------------------------------

def numpy_agg(sched, src_tables, region_tables, core):
    """src_tables: full source table [rows, F] (np); region_tables: list of zeroed np arrays."""
    F = 64
    for ch in sched.chunks:
        cols = ch["cols"]
        idx = _unwrap(np.concatenate(sched.idx[core], axis=1), ch["idx_off"], ch["n_slots"])
        sc = np.concatenate(sched.scale[core], axis=1)[:, ch["scale_off"]:ch["scale_off"] + cols]
        g = src_tables[ch["src_off"] + idx.reshape(cols, P)]  # [cols, P, F] slot=(c*128+p)
        g = g.transpose(1, 0, 2) * sc[:, :, None]             # [P, cols, F]
        stage = np.zeros((P, ch["nw"], F), np.float32)
        wi = 0
        for (c0, nw, D) in ch["runs"]:
            for k in range(nw):
                stage[:, wi] = g[:, c0 + k * D:c0 + (k + 1) * D].sum(axis=1)
                wi += 1
        sidx = _unwrap(np.concatenate(sched.sidx[core], axis=1), ch["sidx_off"], ch["n_rows"])
        rt = region_tables[ch["region"]]
        for i in range(ch["n_rows"]):
            r = sidx[i]
            rt[r] += stage[i % P, i // P]


def _unwrap(stream, off, n):
    blk = stream[:16, off:off + n // 16]
    return blk.T.reshape(-1)[:n].astype(np.int64)


# ----------------------------------------------------------------------------
# Bass kernel builder
# ----------------------------------------------------------------------------

def build_nc(pre, AD, NLAYERS=3, HD=128):
    import concourse.bass as bass
    import concourse.mybir as mybir
    from concourse import bacc
    from concourse.tile import TileContext
    from concourse.masks import make_identity

    F = 64
    AF = mybir.ActivationFunctionType
    OP = mybir.AluOpType
    f32 = mybir.dt.float32
    i16 = mybir.dt.int16

    N = pre["N"]; G = pre["G"]
    NPAD = ((N + 255) // 256) * 256
    NT = NPAD // P                      # node tiles
    NCH = NT // 2                       # apply chunks (2 tiles each)
    EH = pre["EH"]; NQ = pre["NQ"]
    EF_ROWS = ((2 * EH + 127) // 128) * 128
    R1 = ((pre["p1_region_size"] + 127) // 128) * 128
    R2 = ((pre["p2_region_size"] + 127) // 128) * 128
    NR1 = pre["n_p1_regions"]; NR2 = pre["n_p2_regions"]
    s1, s2 = pre["sched1"], pre["sched2"]
    GW = (G + P - 1) // P               # pooled windows

    nc = bacc.Bacc(num_devices=NCORES)
    # ---- inputs
    xT = nc.dram_tensor("xT", [AD, NPAD], f32, kind="ExternalInput")
    W_emb = nc.dram_tensor("W_emb", [AD, F], f32, kind="ExternalInput")
    b_emb = nc.dram_tensor("b_emb", [1, F], f32, kind="ExternalInput")
    convW2 = nc.dram_tensor("convW2", [NLAYERS * P, F], f32, kind="ExternalInput")
    conv_bc = nc.dram_tensor("conv_bc", [F, NLAYERS], f32, kind="ExternalInput")
    bn_gc = nc.dram_tensor("bn_gc", [F, NLAYERS], f32, kind="ExternalInput")
    bn_bc = nc.dram_tensor("bn_bc", [F, NLAYERS], f32, kind="ExternalInput")
    fc_W = nc.dram_tensor("fc_W", [F, HD], f32, kind="ExternalInput")
    fc_b = nc.dram_tensor("fc_b", [1, HD], f32, kind="ExternalInput")
    fco_W = nc.dram_tensor("fco_W", [HD, 1], f32, kind="ExternalInput")
    fco_b = nc.dram_tensor("fco_b", [1, 1], f32, kind="ExternalInput")
    p1_idx = nc.dram_tensor("p1_idx", [P, max(s1.idx_cols, 1)], i16, kind="ExternalInput")
    p1_sc = nc.dram_tensor("p1_sc", [P, max(s1.scale_cols, 1)], f32, kind="ExternalInput")
    p1_sx = nc.dram_tensor("p1_sx", [P, max(s1.sidx_cols, 1)], i16, kind="ExternalInput")
    p2_idx = nc.dram_tensor("p2_idx", [P, max(s2.idx_cols, 1)], i16, kind="ExternalInput")
    p2_sc = nc.dram_tensor("p2_sc", [P, max(s2.scale_cols, 1)], f32, kind="ExternalInput")
    p2_sx = nc.dram_tensor("p2_sx", [P, max(s2.sidx_cols, 1)], i16, kind="ExternalInput")
    PCOLS = pre["pool_data"].shape[1]
    pool_d = nc.dram_tensor("pool_d", [P, PCOLS], f32, kind="ExternalInput")
    # ---- internal tables
    HTAB = nc.dram_tensor("HTAB", [NPAD, F], f32)
    EF = nc.dram_tensor("EF", [EF_ROWS, F], f32)
    REG1 = [nc.dram_tensor(f"REG1_{l}_{r}", [R1, F], f32)
            for l in range(NLAYERS) for r in range(NR1)]
    REG2 = [nc.dram_tensor(f"REG2_{l}_{r}", [R2, F], f32)
            for l in range(NLAYERS) for r in range(NR2)]
    ARIN = nc.dram_tensor("ARIN", [NPAD, F], f32)
    ARO = nc.dram_tensor("ARO", [NPAD, F], f32, addr_space="Shared")
    OUT = nc.dram_tensor("OUT", [GW * P, 1], f32, kind="ExternalOutput")

    KB = int(os.environ.get("KBISECT", "9"))
    with TileContext(nc) as tc:
        with (
            tc.tile_pool(name="const", bufs=1) as cpool,
            tc.tile_pool(name="gbuf", bufs=3) as gpool,
            tc.tile_pool(name="stage", bufs=3) as spool,
            tc.tile_pool(name="small", bufs=2) as mpool,
            tc.tile_pool(name="dense", bufs=3) as dpool,
            tc.tile_pool(name="psum", bufs=4, space="PSUM") as ppool,
            tc.tile_pool(name="psA", bufs=1, space="PSUM") as psA,
        ):
            ident = cpool.tile([P, P], f32)
            make_identity(nc, ident[:])
            ones_col = cpool.tile([P, 1], f32)
            nc.gpsimd.memset(ones_col[:], 1.0)
            ones_row = cpool.tile([1, P], f32)
            nc.gpsimd.memset(ones_row[:], 1.0)
            zt = cpool.tile([P, 32, F], f32)
            nc.gpsimd.memset(zt[:], 0.0)

            # ---------- zero all regions + ARIN pad rows
            def zero_table(tab, rows):
                r3 = tab[:, :].rearrange("(k p) f -> p k f", p=P)
                K = rows // P
                for k0 in range(0, K, 32):
                    kk = min(32, K - k0)
                    nc.sync.dma_start(out=r3[:, k0:k0 + kk, :], in_=zt[:, :kk, :])
            for t in REG1:
                zero_table(t, R1)
            for t in REG2:
                zero_table(t, R2)
            zero_table(ARIN, NPAD)

            # ---------- embedding: HTAB = x @ W_emb + b_emb
            wemb_t = cpool.tile([AD, F], f32)
            nc.sync.dma_start(out=wemb_t[:], in_=W_emb[:, :])
            bemb_t = cpool.tile([1, F], f32)
            nc.sync.dma_start(out=bemb_t[:], in_=b_emb[:, :])
            # broadcast b_emb to [128, 256] via PE: ones_row^T @ bemb
            bps = ppool.tile([P, F], f32, space="PSUM", tag="ps")
            nc.tensor.matmul(out=bps[:], lhsT=ones_row[:], rhs=bemb_t[:],
                             start=True, stop=True)
            bemb4 = cpool.tile([P, 4, F], f32)
            for j in range(4):
                nc.vector.tensor_copy(out=bemb4[:, j, :], in_=bps[:])
            h3 = HTAB[:, :].rearrange("(t p) f -> p t f", p=P)
            for t0 in range(0, NT, 4):
                tt = min(4, NT - t0)
                eps_ = ppool.tile([P, 4 * F], f32, space="PSUM", tag="ps")
                for j in range(tt):
                    xc = gpool.tile([AD, P], f32)
                    nc.sync.dma_start(out=xc[:], in_=xT[:, (t0 + j) * P:(t0 + j + 1) * P])
                    nc.tensor.matmul(out=eps_[:, j * F:(j + 1) * F], lhsT=xc[:],
                                     rhs=wemb_t[:], start=True, stop=True)
                hb = spool.tile([P, 4, F], f32)
                nc.vector.tensor_tensor(out=hb[:, :tt, :],
                                        in0=eps_[:, :tt * F].rearrange("p (t f) -> p t f", f=F),
                                        in1=bemb4[:, :tt, :], op=OP.add)
                nc.sync.dma_start(out=h3[:, t0:t0 + tt, :], in_=hb[:, :tt, :])

            # ---------- layers
            ef3 = EF[:, :].rearrange("(t p) f -> p t f", p=P)
            ar3 = ARIN[:, :].rearrange("(t p) f -> p t f", p=P)
            for l in range(NLAYERS if KB >= 8 else (1 if KB >= 1 else 0)):
                # ===== P1: gather HTAB -> piece regions
                def agg_pass(sched, idx_d, sc_d, sx_d, src3, src_rows, regs, use_scale):
                    for ch in sched.chunks:
                        cols = ch["cols"]; nw = ch["nw"]
                        it = gpool.tile([P, MAXNI // 16], i16)
                        nc.sync.dma_start(
                            out=it[:, :ch["n_slots"] // 16],
                            in_=idx_d[:, ch["idx_off"]:ch["idx_off"] + ch["n_slots"] // 16])
                        g = gpool.tile([P, MAXCOLS, F], f32)
                        hi_ = min(ch["src_off"] + 32760, src_rows)
                        nc.gpsimd.dma_gather(
                            out_ap=g[:, :cols, :], in_ap=src3[ch["src_off"]:hi_, :],
                            idxs_ap=it[:, :ch["n_slots"] // 16],
                            num_idxs=ch["n_slots"], num_idxs_reg=ch["n_slots"], elem_size=F)
                        if use_scale:
                            st = gpool.tile([P, MAXCOLS], f32)
                            nc.sync.dma_start(
                                out=st[:, :cols],
                                in_=sc_d[:, ch["scale_off"]:ch["scale_off"] + cols])
                            gs = gpool.tile([P, MAXCOLS, F], f32)
                            nc.vector.tensor_tensor(
                                out=gs[:, :cols, :], in0=g[:, :cols, :],
                                in1=st[:, :cols].to_broadcast([P, cols, F]), op=OP.mult)
                        else:
                            gs = g
                        stg = spool.tile([P, MAXCOLS, F], f32)
                        wi = 0
                        for (c0, rnw, D) in ch["runs"]:
                            if D == 1:
                                nc.vector.tensor_copy(out=stg[:, wi:wi + rnw, :],
                                                      in_=gs[:, c0:c0 + rnw, :])
                            else:
                                src = gs[:, c0:c0 + rnw * D, :].rearrange(
                                    "p (w d) f -> p w f d", w=rnw, d=D)
                                nc.vector.tensor_reduce(out=stg[:, wi:wi + rnw, :], in_=src,
                                                        axis=mybir.AxisListType.X, op=OP.add)
                            wi += rnw
                        xt_ = gpool.tile([P, MAXNI // 16], i16)
                        nc.sync.dma_start(
                            out=xt_[:, :ch["n_rows"] // 16],
                            in_=sx_d[:, ch["sidx_off"]:ch["sidx_off"] + ch["n_rows"] // 16])
                        nc.gpsimd.dma_scatter_add(
                            out_ap=regs[ch["region"]][:, :],
                            in_ap=stg[:, :nw, :], idxs_ap=xt_[:, :ch["n_rows"] // 16],
                            num_idxs=ch["n_rows"], num_idxs_reg=ch["n_rows"], elem_size=F)

                agg_pass(s1, p1_idx, p1_sc, p1_sx, HTAB, NPAD, REG1[l * NR1:(l + 1) * NR1], True)
                if KB == 1: continue

                # ===== L2-P1: EF[h*EH + r] = sum_b REG1[(b,h)][r]
                nblk = pre["nblk"]
                for h in range(2):
                    rh = [REG1[l * NR1 + b * 2 + h] for b in range(nblk)]
                    KH = (EH + 127) // 128  # tiles of 128 rows (region rows >= EH)
                    for k0 in range(0, KH, 8):
                        kk = min(8, KH - k0)
                        mt = spool.tile([P, 4, 8, F], f32, tag="mt")
                        if nblk < 4:
                            nc.gpsimd.memset(mt[:], 0.0)
                        for b in range(nblk):
                            rb3 = rh[b][:, :].rearrange("(t p) f -> p t f", p=P)
                            nc.sync.dma_start(out=mt[:, b, :kk, :], in_=rb3[:, k0:k0 + kk, :])
                        red = spool.tile([P, 8, F], f32, tag="red")
                        src = mt[:, :, :kk, :].rearrange("p b w f -> p w f b")
                        nc.vector.tensor_reduce(out=red[:, :kk, :], in_=src,
                                                axis=mybir.AxisListType.X, op=OP.add)
                        # EF rows h*EH + (k*128+p): not 128-aligned for h=1 in general ->
                        # EF laid out as [2, EHP] with EHP=ceil(EH/128)*128
                        EHP = ((EH + 127) // 128) * 128
                        off = h * EHP + k0 * P
                        nc.sync.dma_start(
                            out=EF[off:off + kk * P, :].rearrange("(t p) f -> p t f", p=P),
                            in_=red[:, :kk, :])

                if KB == 2: continue
                # ===== P2: gather EF -> partial piece regions (scale = Binv*Dinv)
                agg_pass(s2, p2_idx, p2_sc, p2_sx, EF, EF_ROWS, REG2[l * NR2:(l + 1) * NR2], True)

                if KB == 3: continue
                # ===== L2-P2: ARIN[n] = sum_h REG2[(h, q)][n - q*NQ]
                for q in range(4):
                    lo = q * NQ
                    hi = min(lo + NQ, N)
                    rows = hi - lo
                    KH = (rows + 127) // 128
                    for k0 in range(0, KH, 8):
                        kk = min(8, KH - k0)
                        mt = spool.tile([P, 2, 8, F], f32, tag="mt")
                        for h in range(2):
                            rb3 = REG2[l * NR2 + h * 4 + q][:, :].rearrange(
                                "(t p) f -> p t f", p=P)
                            nc.sync.dma_start(out=mt[:, h, :kk, :], in_=rb3[:, k0:k0 + kk, :])
                        red = spool.tile([P, 8, F], f32, tag="red")
                        src = mt[:, :, :kk, :].rearrange("p b w f -> p w f b")
                        nc.vector.tensor_reduce(out=red[:, :kk, :], in_=src,
                                                axis=mybir.AxisListType.X, op=OP.add)
                        # ARIN rows lo + k0*128 ... may exceed hi on last block; host
                        # guarantees NQ % 128 == 0 except last quarter; clamp rows:
                        wlim = min(kk * P, rows - k0 * P)
                        full_w = wlim // P
                        if full_w > 0:
                            nc.sync.dma_start(
                                out=ARIN[lo + k0 * P: lo + k0 * P + full_w * P, :]
                                    .rearrange("(t p) f -> p t f", p=P),
                                in_=red[:, :full_w, :])
                        remp = wlim - full_w * P
                        if remp > 0:
                            nc.sync.dma_start(
                                out=ARIN[lo + (k0 + full_w) * P: lo + (k0 + full_w) * P + remp, :],
                                in_=red[:remp, full_w, :])

                if KB == 4: continue
                # ===== AllReduce
                nc.gpsimd.collective_compute(
                    "AllReduce", OP.add, replica_groups=[list(range(NCORES))],
                    ins=[ARIN[:, :]],
                    outs=[ARO[:, :]])

                if KB == 5: continue
                # KB>=6: dense runs
                # ===== stats: C = raw2^T raw2, S1 = raw2^T 1  (over N real rows)
                o3 = ARO[:, :].rearrange("(t p) f -> p t f", p=P)
                Cps = psA.tile([F, F], f32, space="PSUM", tag="C")
                Sps = psA.tile([F, 1], f32, space="PSUM", tag="S")
                NTF = (N + P - 1) // P      # 782 tiles; last has N - (NTF-1)*128 rows
                lastp = N - (NTF - 1) * P
                for t in range(NTF):
                    rt = dpool.tile([P, F], f32)
                    pp = P if t < NTF - 1 else lastp
                    nc.sync.dma_start(out=rt[:pp, :], in_=o3[:pp, t, :])
                    nc.tensor.matmul(out=Cps[:], lhsT=rt[:pp, :], rhs=rt[:pp, :],
                                     start=(t == 0), stop=(t == NTF - 1))
                    nc.tensor.matmul(out=Sps[:], lhsT=rt[:pp, :], rhs=ones_col[:pp, :],
                                     start=(t == 0), stop=(t == NTF - 1))

                if KB == 60: continue
                # ===== postprocess: a, cfin, Wp2, cb2
                Wl = cpool.tile([P, F], f32, tag="Wl")
                nc.sync.dma_start(out=Wl[:], in_=convW2[l * P:(l + 1) * P, :])
                bcol = mpool.tile([F, 1], f32)
                nc.sync.dma_start(out=bcol[:], in_=conv_bc[:, l:l + 1])
                gcol = mpool.tile([F, 1], f32)
                nc.sync.dma_start(out=gcol[:], in_=bn_gc[:, l:l + 1])
                btcol = mpool.tile([F, 1], f32)
                nc.sync.dma_start(out=btcol[:], in_=bn_bc[:, l:l + 1])
                Cs = mpool.tile([F, F], f32)
                nc.vector.tensor_copy(out=Cs[:], in_=Cps[:])
                Ss = mpool.tile([F, 1], f32)
                nc.vector.tensor_copy(out=Ss[:], in_=Sps[:])
                mps = ppool.tile([F, 1], f32, space="PSUM", tag="ps")
                nc.tensor.matmul(out=mps[:], lhsT=Wl[:F, :], rhs=Ss[:], start=True, stop=True)
                mrw = mpool.tile([F, 1], f32)
                nc.scalar.activation(out=mrw[:], in_=mps[:], func=AF.Copy, scale=1.0 / N)
                t1ps = ppool.tile([F, F], f32, space="PSUM", tag="ps")
                nc.tensor.matmul(out=t1ps[:], lhsT=Cs[:], rhs=Wl[:F, :], start=True, stop=True)
                wt1 = mpool.tile([F, F], f32)
                nc.vector.tensor_tensor(out=wt1[:], in0=t1ps[:], in1=Wl[:F, :], op=OP.mult)
                e2ps = ppool.tile([F, 1], f32, space="PSUM", tag="ps")
                nc.tensor.matmul(out=e2ps[:], lhsT=wt1[:], rhs=ones_col[:F, :], start=True, stop=True)
                var = mpool.tile([F, 1], f32)
                nc.scalar.activation(out=var[:], in_=e2ps[:], func=AF.Copy, scale=1.0 / N)
                msq = mpool.tile([F, 1], f32)
                nc.vector.tensor_tensor(out=msq[:], in0=mrw[:], in1=mrw[:], op=OP.mult)
                nc.vector.tensor_tensor(out=var[:], in0=var[:], in1=msq[:], op=OP.subtract)
                nc.vector.tensor_scalar_add(out=var[:], in0=var[:], scalar1=1e-5)
                lnv = mpool.tile([F, 1], f32)
                nc.scalar.activation(out=lnv[:], in_=var[:], func=AF.Ln)
                rstd = mpool.tile([F, 1], f32)
                nc.scalar.activation(out=rstd[:], in_=lnv[:], func=AF.Exp, scale=-0.5)
                a_ = mpool.tile([F, 1], f32)
                nc.vector.tensor_tensor(out=a_[:], in0=gcol[:], in1=rstd[:], op=OP.mult)
                am = mpool.tile([F, 1], f32)
                nc.vector.tensor_tensor(out=am[:], in0=a_[:], in1=mrw[:], op=OP.mult)
                cfin = mpool.tile([F, 1], f32)
                nc.vector.tensor_tensor(out=cfin[:], in0=btcol[:], in1=am[:], op=OP.subtract)
                # rows
                arps = ppool.tile([1, F], f32, space="PSUM", tag="ps")
                nc.tensor.matmul(out=arps[:], lhsT=a_[:], rhs=ident[:F, :F], start=True, stop=True)
                arow = mpool.tile([1, F], f32)
                nc.vector.tensor_copy(out=arow[:], in_=arps[:])
                crps = ppool.tile([1, F], f32, space="PSUM", tag="ps")
                nc.tensor.matmul(out=crps[:], lhsT=cfin[:], rhs=ident[:F, :F], start=True, stop=True)
                crow = mpool.tile([1, F], f32)
                nc.vector.tensor_copy(out=crow[:], in_=crps[:])
                abps = ppool.tile([P, F], f32, space="PSUM", tag="ps")
                nc.tensor.matmul(out=abps[:], lhsT=ones_row[:], rhs=arow[:], start=True, stop=True)
                Wp2 = cpool.tile([P, F], f32, tag="Wp")
                nc.vector.tensor_tensor(out=Wp2[:], in0=Wl[:], in1=abps[:], op=OP.mult)
                cbps = ppool.tile([P, F], f32, space="PSUM", tag="ps")
                nc.tensor.matmul(out=cbps[:], lhsT=ones_row[:], rhs=crow[:], start=True, stop=True)
                cb2 = cpool.tile([P, 2, F], f32, tag="cb")
                nc.vector.tensor_copy(out=cb2[:, 0, :], in_=cbps[:])
                nc.vector.tensor_copy(out=cb2[:, 1, :], in_=cbps[:])

                if KB == 61: continue
                # ===== apply: HTAB = softplus(ARO @ Wp2 + cb2); fused pooling on l==2
                if (KB not in (60, 61, 62)) and l == (NLAYERS - 1 if KB >= 8 else 0):
                    poolacc = cpool.tile([F, GW * P], f32)
                    nc.gpsimd.memset(poolacc[:], 0.0)
                    poold_t = cpool.tile([P, PCOLS], f32)
                    nc.sync.dma_start(out=poold_t[:], in_=pool_d[:, :])
                    pm_by_tile = {}
                    for m in pre["pool_meta"]:
                        pm_by_tile.setdefault(m["tile"], []).append(m)
                for c in range(NCH):
                    rc = dpool.tile([P, 2, F], f32)
                    nc.sync.dma_start(out=rc[:], in_=o3[:, 2 * c:2 * c + 2, :])
                    yps = ppool.tile([P, 2 * F], f32, space="PSUM", tag="ps")
                    for j in (0, 1):
                        trp = ppool.tile([F, P], f32, space="PSUM", tag="ps")
                        nc.tensor.transpose(out=trp[:], in_=rc[:, j, :], identity=ident[:])
                        trs = dpool.tile([F, P], f32, tag="trs")
                        nc.vector.tensor_copy(out=trs[:], in_=trp[:])
                        nc.tensor.matmul(out=yps[:, j * F:(j + 1) * F], lhsT=trs[:],
                                         rhs=Wp2[0:F, :], start=True, stop=True)
                    yb = dpool.tile([P, 2 * F], f32)
                    nc.vector.tensor_tensor(out=yb[:],
                                            in0=yps[:],
                                            in1=cb2[:].rearrange("p t f -> p (t f)"), op=OP.add)
                    ex = dpool.tile([P, 2 * F], f32)
                    nc.scalar.activation(out=ex[:], in_=yb[:], func=AF.Exp)
                    hc = dpool.tile([P, 2, F], f32)
                    nc.scalar.activation(out=hc[:].rearrange("p t f -> p (t f)"), in_=ex[:],
                                         func=AF.Ln, bias=1.0, scale=1.0)
                    nc.sync.dma_start(out=h3[:, 2 * c:2 * c + 2, :], in_=hc[:])
                    if (KB not in (60, 61, 62)) and l == (NLAYERS - 1 if KB >= 8 else 0):
                        for j in (0, 1):
                            for m in pm_by_tile.get(2 * c + j, []):
                                pps = ppool.tile([F, P], f32, space="PSUM", tag="ps")
                                nc.tensor.matmul(
                                    out=pps[:, :m["ncol"]],
                                    lhsT=hc[:, j, :],
                                    rhs=poold_t[:, m["col_off"]:m["col_off"] + m["ncol"]],
                                    start=True, stop=True)
                                go = m["window"] * P + m["grow"]
                                nc.vector.tensor_tensor(
                                    out=poolacc[:, go:go + m["ncol"]],
                                    in0=poolacc[:, go:go + m["ncol"]],
                                    in1=pps[:, :m["ncol"]], op=OP.add)

            # ---------- FC head
            if KB < 6 or KB in (60, 61, 62):
                dz = dpool.tile([P, 1], f32)
                nc.gpsimd.memset(dz[:], 0.0)
                for w in range(GW):
                    nc.sync.dma_start(out=OUT[w * P:(w + 1) * P, :], in_=dz[:])
                poolacc = None
            fcw_t = cpool.tile([F, HD], f32)
            nc.sync.dma_start(out=fcw_t[:], in_=fc_W[:, :])
            if KB < 6 or KB in (60, 61, 62):
                poolacc = None
            fcb_t = mpool.tile([1, HD], f32)
            nc.sync.dma_start(out=fcb_t[:], in_=fc_b[:, :])
            fcbps = ppool.tile([P, HD], f32, space="PSUM", tag="ps")
            nc.tensor.matmul(out=fcbps[:], lhsT=ones_row[:], rhs=fcb_t[:], start=True, stop=True)
            fcb_b = cpool.tile([P, HD], f32)
            nc.vector.tensor_copy(out=fcb_b[:], in_=fcbps[:])
            fcow_t = cpool.tile([HD, 1], f32)
            nc.sync.dma_start(out=fcow_t[:], in_=fco_W[:, :])
            fcob_t = mpool.tile([1, 1], f32)
            nc.sync.dma_start(out=fcob_t[:], in_=fco_b[:, :])
            fcobps = ppool.tile([P, 1], f32, space="PSUM", tag="ps")
            nc.tensor.matmul(out=fcobps[:], lhsT=ones_row[:], rhs=fcob_t[:], start=True, stop=True)
            fcob_b = cpool.tile([P, 1], f32)
            nc.vector.tensor_copy(out=fcob_b[:], in_=fcobps[:])
            for w in range(GW if (KB >= 6 and KB not in (60, 61, 62)) else 0):
                gts = dpool.tile([F, P], f32)
                ge_ = dpool.tile([F, P], f32)
                nc.scalar.activation(out=ge_[:], in_=poolacc[:, w * P:(w + 1) * P], func=AF.Exp)
                nc.scalar.activation(out=gts[:], in_=ge_[:], func=AF.Ln, bias=1.0, scale=1.0)
                g2ps = ppool.tile([P, HD], f32, space="PSUM", tag="ps")
                nc.tensor.matmul(out=g2ps[:], lhsT=gts[:], rhs=fcw_t[:], start=True, stop=True)
                g2b = dpool.tile([P, HD], f32)
                nc.vector.tensor_tensor(out=g2b[:], in0=g2ps[:], in1=fcb_b[:], op=OP.add)
                g2e = dpool.tile([P, HD], f32)
                nc.scalar.activation(out=g2e[:], in_=g2b[:], func=AF.Exp)
                g2 = dpool.tile([P, HD], f32)
                nc.scalar.activation(out=g2[:], in_=g2e[:], func=AF.Ln, bias=1.0, scale=1.0)
                g2tp = ppool.tile([P, P], f32, space="PSUM", tag="ps")
                nc.tensor.transpose(out=g2tp[:], in_=g2[:], identity=ident[:])
                g2ts = dpool.tile([P, P], f32)
                nc.vector.tensor_copy(out=g2ts[:], in_=g2tp[:])
                y_ps = ppool.tile([P, 1], f32, space="PSUM", tag="ps")
                nc.tensor.matmul(out=y_ps[:], lhsT=g2ts[:], rhs=fcow_t[:], start=True, stop=True)
                y_t = dpool.tile([P, 1], f32)
                nc.vector.tensor_tensor(out=y_t[:], in0=y_ps[:], in1=fcob_b[:], op=OP.add)
                nc.sync.dma_start(out=OUT[w * P:(w + 1) * P, :], in_=y_t[:])
    return nc


def _streams(pre, core):
    s1, s2 = pre["sched1"], pre["sched2"]
    def cat(lst, dtype, n):
        if not lst:
            return np.zeros((P, 1), dtype)
        a = np.concatenate(lst, axis=1).astype(dtype)
        assert a.shape[1] == n, (a.shape, n)
        return a
    return dict(
        p1_idx=cat(s1.idx[core], np.int16, s1.idx_cols),
        p1_sc=cat(s1.scale[core], np.float32, s1.scale_cols),
        p1_sx=cat(s1.sidx[core], np.int16, s1.sidx_cols),
        p2_idx=cat(s2.idx[core], np.int16, s2.idx_cols),
        p2_sc=cat(s2.scale[core], np.float32, s2.scale_cols),
        p2_sx=cat(s2.sidx[core], np.int16, s2.sidx_cols),
    )


def prepare(x, W_emb, b_emb, conv_W, conv_b, bn_gamma, bn_beta,
            fc_W, fc_b, fco_W, fco_b, node_idx, edge_idx, batch):
    """Build (nc, in_maps, G) — the finalized Bass module and per-core inputs."""
    x = np.asarray(x, np.float32)
    N, AD = x.shape
    G = int(np.asarray(batch).max()) + 1
    pre = host_prep(x, np.asarray(node_idx, np.int64), np.asarray(edge_idx, np.int64),
                    np.asarray(batch, np.int64))
    NL = np.asarray(conv_W).shape[0]
    nc = build_nc(pre, AD, NLAYERS=NL, HD=np.asarray(fc_W).shape[1])
    nc.finalize()
    NPAD = ((N + 255) // 256) * 256
    xTp = np.zeros((AD, NPAD), np.float32)
    xTp[:, :N] = x.T
    convW2 = np.concatenate([np.concatenate([w, w], axis=0) for w in np.asarray(conv_W, np.float32)], axis=0)
    common = dict(
        xT=xTp, W_emb=np.asarray(W_emb, np.float32),
        b_emb=np.asarray(b_emb, np.float32).reshape(1, -1),
        convW2=convW2,
        conv_bc=np.asarray(conv_b, np.float32).T.copy(),
        bn_gc=np.asarray(bn_gamma, np.float32).T.copy(),
        bn_bc=np.asarray(bn_beta, np.float32).T.copy(),
        fc_W=np.asarray(fc_W, np.float32),
        fc_b=np.asarray(fc_b, np.float32).reshape(1, -1),
        fco_W=np.asarray(fco_W, np.float32),
        fco_b=np.asarray(fco_b, np.float32).reshape(1, 1),
        pool_d=pre["pool_data"].astype(np.float32),
    )
    in_maps = []
    for c in range(NCORES):
        m = dict(common)
        m.update(_streams(pre, c))
        in_maps.append(m)
    return nc, in_maps, G


def kernel(x, W_emb, b_emb, conv_W, conv_b, bn_gamma, bn_beta,
           fc_W, fc_b, fco_W, fco_b, node_idx, edge_idx, batch, use_sim=False):
    from concourse.bass_utils import run_bass_kernel_spmd
    nc, in_maps, G = prepare(x, W_emb, b_emb, conv_W, conv_b, bn_gamma, bn_beta,
                             fc_W, fc_b, fco_W, fco_b, node_idx, edge_idx, batch)
    if use_sim:
        from concourse.bass_interp import MultiCoreSim
        sim = MultiCoreSim(nc, num_cores=NCORES, num_workers=8)
        for cid, cs in sim.cores.items():
            for name, arr in in_maps[cid].items():
                cs.tensor(name)[:] = arr
        sim.simulate()
        out = np.array(sim.cores[0].tensor("OUT"))[:G]
        return out.astype(np.float32)
    res = run_bass_kernel_spmd(nc, in_maps, core_ids=list(range(NCORES)), trace=False)
    global LAST_EXEC_NS
    LAST_EXEC_NS = res.exec_time_ns
    out = res.results[0]["OUT"][:G]
    return out.astype(np.float32)



# revision 48
# speedup vs baseline: 10.6483x; 10.6483x over previous
"""CHGCNN hypergraph-conv forward on 8 Trainium2 NeuronCores (Bass/Tile).

Strategy (per core, SPMD single NEFF, 4 SWDGE queues):
  - Edges sharded across 8 cores (dealt round-robin by degree). Nodes canonical 0..N-1.
  - Layer l: h table HTAB [N,64] replicated per core (built by AllGather of slices).
    P1: e_feat_piece[(e,src_block)] = sum of HTAB[node] over pairs (dma_gather chunks
        + per-slot scale + strided DVE window reduce + dma_scatter_add of unique piece
        rows into per-(srcblk, dsthalf) regions; chunks interleaved across regions and
        rotated over 4 SWDGE queues so scatter DMAs overlap; idx/scale/sidx streams
        block-loaded 8 chunks per DMA).
    L2-P1: e_feat = sum of 4 piece regions (dense reads + strided DVE reduce);
        regions re-zeroed for the next layer during the dense phase.
    P2: partial[(n, e_half)] pieces from gathers of e_feat (scale = Binv*Dinv),
        scattered into per-(e_half, n_quarter) regions; L2-P2: ARIN[n] = sum of halves.
    ReduceScatter(ARIN) -> per-core node slice RSO [N/8, 64].
    Dense on the slice: stats C=raw2^T raw2, S=raw2^T 1 (98 tiles) -> tiny AllReduce of
    [C;S^T]; BN folded into W'=W*diag(a), c'=a*b+beta-a*mean; apply h'=softplus(raw2@W'+c')
    via PE-transpose+matmul; layers 0..L-2: AllGather slices -> HTAB.
  - Last layer: no AllGather; per-graph mean pooling = slice segment-sum by graph through
    the same gather/reduce/scatter machinery on HSL + AllReduce of [512,64] partials;
    FC stack on every core; out [512,1] (identical on cores).
"""
import sys, os
sys.path.insert(0, "/opt/trn_rl_repo")
import numpy as np

class _KBDone(Exception):
    pass

LAST_EXEC_NS = None

P = 128
NCORES = 8
MAXNI = 1024          # validated dma_gather/scatter max rows per instruction
MAXCOLS = MAXNI // P  # 8 columns per chunk
BLK = 25000           # node table block size for int16 gathers (<32768)

# ----------------------------------------------------------------------------
# Host schedule construction
# ----------------------------------------------------------------------------

def _wrap16(v):
    """dma_gather/scatter idx layout: slot i -> [i%16, i//16], replicated to 128 rows."""
    n = len(v)
    assert n % 16 == 0
    w = np.asarray(v, np.int16).reshape(n // 16, 16).T
    return np.tile(w, (8, 1))  # [128, n//16]


class AggSched:
    """Uniform (cross-core) schedule for one gather->reduce->scatter pass.

    Per core data streams: idx (int16 wrapped), scale (fp32 [128, cols]),
    sidx (int16 wrapped scatter rows).
    chunks: list of (src_row_off, n_slots, idx_coloff, scale_coloff,
                     runs=[(col0, nw, D)], region, n_rows, sidx_off, serial)
    """
    def __init__(self):
        self.chunks = []
        self.idx = [[] for _ in range(NCORES)]     # per-core list of [128, k] blocks
        self.scale = [[] for _ in range(NCORES)]   # per-core list of [128, c] blocks
        self.sidx = [[] for _ in range(NCORES)]
        self.idx_cols = 0
        self.scale_cols = 0
        self.sidx_cols = 0


def build_agg(per_core_segs, src_block_of, src_off_of, n_regions, region_rows):
    """per_core_segs: dict key -> list over cores of lists of
         (dst_region, dst_row, [(src_row_global, scale), ...])
       key must include (region, src_block); all segs of a key share both.
       src_block_of/src_off_of: key -> src block row offset (for gather in_ap)
       Region dump row = region_rows[r] (scatter pad target).
    Returns AggSched. Segments are padded across cores for SPMD uniformity.
    """
    s = AggSched()
    for key in sorted(per_core_segs.keys()):
        percore = per_core_segs[key]
        region = None
        nseg = max(len(x) for x in percore)
        if nseg == 0:
            continue
        # sort by count desc per core
        percore = [sorted(x, key=lambda t: -len(t[2])) for x in percore]
        src_off = src_off_of(key)
        # windows of 128 segments
        nwin = (nseg + P - 1) // P
        w0 = 0
        pend_windows = []  # (D, per-core [(dst_row, srcs, scales)] x128)
        for w in range(nwin):
            lo = w * P
            D = 0
            for c in range(NCORES):
                if lo < len(percore[c]):
                    D = max(D, len(percore[c][lo][2]))
            D = max(D, 1)
            win = []
            for c in range(NCORES):
                rows = []
                for p in range(P):
                    i = lo + p
                    if i < len(percore[c]):
                        reg, dst, pairs = percore[c][i]
                        rows.append((dst, pairs))
                    else:
                        rows.append((None, []))
                win.append(rows)
            region = key[0]
            pend_windows.append((D, win))
        # chunk windows: sum of D <= MAXCOLS
        i = 0
        while i < len(pend_windows):
            j = i
            tot = 0
            while j < len(pend_windows) and tot + pend_windows[j][0] <= MAXCOLS:
                tot += pend_windows[j][0]
                j += 1
            group = pend_windows[i:j]
            i = j
            # emit chunk
            cols = sum(d for d, _ in group)
            nw = len(group)
            idx_block = np.zeros((NCORES, cols * P), np.int64)
            sc_block = np.zeros((NCORES, P, cols), np.float32)
            sidx_block = np.zeros((NCORES, nw * P), np.int64)
            runs = []
            col0 = 0
            rr = region_rows[key[0]]
            for wi, (D, win) in enumerate(group):
                # merge equal-D consecutive into runs
                if runs and runs[-1][2] == D and runs[-1][0] + runs[-1][1] * runs[-1][2] == col0:
                    runs[-1] = (runs[-1][0], runs[-1][1] + 1, D)
                else:
                    runs.append((col0, 1, D))
                for c in range(NCORES):
                    for p in range(P):
                        dst, pairs = win[c][p]
                        sidx_block[c, wi * P + p] = rr if dst is None else dst
                        for t in range(D):
                            col = col0 + t
                            slot = col * P + p
                            if t < len(pairs):
                                sr, sv = pairs[t]
                                idx_block[c, slot] = sr - src_off
                                sc_block[c, p, col] = sv
                            else:
                                idx_block[c, slot] = 0
                                sc_block[c, p, col] = 0.0
                col0 += D
            for c in range(NCORES):
                s.idx[c].append(_wrap16(idx_block[c]))
                s.scale[c].append(sc_block[c])
                s.sidx[c].append(_wrap16(sidx_block[c]))
            s.chunks.append(dict(
                src_off=src_off, n_slots=cols * P, cols=cols,
                idx_off=s.idx_cols, scale_off=s.scale_cols,
                runs=runs, region=region, n_rows=nw * P, nw=nw,
                sidx_off=s.sidx_cols))
            s.idx_cols += cols * P // 16
            s.scale_cols += cols
            s.sidx_cols += nw * P // 16
    return s


def host_prep(x, node_idx, edge_idx, batch):
    N = x.shape[0]
    I = node_idx.shape[0]
    E = int(edge_idx.max()) + 1 if len(edge_idx) else 1
    G = int(batch.max()) + 1
    node_idx = np.asarray(node_idx, np.int64)
    edge_idx = np.asarray(edge_idx, np.int64)
    batch = np.asarray(batch, np.int64)

    deg_e = np.bincount(edge_idx, minlength=E)
    deg_n = np.bincount(node_idx, minlength=N)
    Binv = np.where(deg_e > 0, 1.0 / np.maximum(deg_e, 1), 0.0).astype(np.float32)
    Dinv = np.where(deg_n > 0, 1.0 / np.maximum(deg_n, 1), 0.0).astype(np.float32)

    # ---- edge shard: live edges dealt by degree
    live = np.nonzero(deg_e > 0)[0]
    order = live[np.argsort(-deg_e[live], kind="stable")]
    core_of_edge = np.full(E, -1, np.int64)
    epos = np.full(E, -1, np.int64)   # local e_feat row
    ii = np.arange(len(order), dtype=np.int64)
    core_of_edge[order] = ii % NCORES
    epos[order] = ii // NCORES
    RE = (len(order) + NCORES - 1) // NCORES if len(order) else 0
    # e_feat halves (128-aligned so EF half offsets match stream indices)
    EH = (((RE + 1) // 2 + 127) // 128) * 128
    assert EH <= 32767

    # pairs grouped per (core)
    pair_core = core_of_edge[edge_idx]

    # ---- P1 segments: key (region=(srcblk, dsthalf)), seg=(dst=piece row, srcs)
    nblk = (N + BLK - 1) // BLK
    p1_regions = {}
    for b in range(nblk):
        for h in range(2):
            p1_regions[(b, h)] = len(p1_regions)
    p1_region_size = EH + 1  # + dump row

    src_blk = node_idx // BLK
    ep_all = epos[edge_idx]
    hh_all = (ep_all >= EH).astype(np.int64)
    erow_all = ep_all - hh_all * EH
    major1 = src_blk * 2 + hh_all          # == p1 region id (b*2+h)
    k1 = (pair_core * (2 * nblk) + major1) * (EH + 1) + erow_all
    o1 = np.argsort(k1, kind="stable")
    ks1 = k1[o1]
    new1 = np.empty(I, bool)
    new1[0] = True
    new1[1:] = ks1[1:] != ks1[:-1]
    st1 = np.nonzero(new1)[0]
    rank1 = np.arange(I) - st1[np.cumsum(new1) - 1]
    lvl1 = np.zeros(I, np.int64)
    lvl1[o1] = rank1 // MAXCOLS
    keys1, arrs1 = _segment_arrays(pair_core, major1, lvl1, erow_all,
                                   node_idx, np.ones(I, np.float32), MAXCOLS)
    sched1 = build_agg_fast(keys1, arrs1, lambda k: (k[0] // 2) * BLK,
                            lambda k: k[0], {r: EH for r in range(2 * nblk)})
    _interleave_chunks(sched1)

    # ---- L2-P1: e_feat[rows RE] = sum over nblk regions at same (h, row)
    # handled densely in kernel (no schedule needed beyond sizes)

    # ---- P2 segments: dst piece (n, e_half) -> region (h, quarter(n))
    NQ = (N + 3) // 4
    assert NQ <= 32767
    p2_regions = {}
    for h in range(2):
        for q in range(4):
            p2_regions[(h, q)] = len(p2_regions)
    p2_region_size = NQ + 1

    sc2 = (Binv[edge_idx] * Dinv[node_idx]).astype(np.float32)
    qq_all = node_idx // NQ
    nrow_all = node_idx - qq_all * NQ
    major2 = hh_all * 4 + qq_all           # == p2 region id (h*4+q)
    src2 = hh_all * EH + erow_all
    k2 = (pair_core * 8 + major2) * (NQ + 1) + nrow_all
    o2 = np.argsort(k2, kind="stable")
    ks2 = k2[o2]
    new2 = np.empty(I, bool)
    new2[0] = True
    new2[1:] = ks2[1:] != ks2[:-1]
    st2 = np.nonzero(new2)[0]
    rank2 = np.arange(I) - st2[np.cumsum(new2) - 1]
    lvl2 = np.zeros(I, np.int64)
    lvl2[o2] = rank2 // MAXCOLS
    keys2, arrs2 = _segment_arrays(pair_core, major2, lvl2, nrow_all,
                                   src2, sc2, MAXCOLS)
    sched2 = build_agg_fast(keys2, arrs2, lambda k: (k[0] // 4) * EH,
                            lambda k: k[0], {r: NQ for r in range(8)})
    _interleave_chunks(sched2)

    # ---- pooling: per-core slice segment-sum by graph (agg machinery)
    cnt_g = np.bincount(batch, minlength=G).astype(np.float32)
    NPADp = ((N + 1023) // 1024) * 1024
    NSs = NPADp // NCORES
    nall = np.arange(N, dtype=np.int64)
    pc_core = np.minimum(nall // NSs, NCORES - 1)
    pscale = (1.0 / np.maximum(cnt_g, 1.0))[batch].astype(np.float32)
    psrc = nall - pc_core * NSs
    pmajor = np.zeros(N, np.int64)
    kp = pc_core * G + batch
    op_ = np.argsort(kp, kind="stable")
    ksp = kp[op_]
    newp = np.empty(N, bool)
    newp[0] = True
    newp[1:] = ksp[1:] != ksp[:-1]
    stp = np.nonzero(newp)[0]
    rankp = np.arange(N) - stp[np.cumsum(newp) - 1]
    lvlp = np.zeros(N, np.int64)
    lvlp[op_] = rankp // MAXCOLS
    keysp, arrsp = _segment_arrays(pc_core, pmajor, lvlp, batch, psrc, pscale, MAXCOLS)
    schedp = build_agg_fast(keysp, arrsp, lambda k: 0, lambda k: 0, {0: G})
    pool_meta = []   # (tile, window, col_lo_graph, ncols, startflags per window handled in build)
    NT = (N + P - 1) // P
    pool_cols = []
    for t in range(NT):
        lo, hi = t * P, min((t + 1) * P, N)
        gs = batch[lo:hi]
        g0, g1 = int(gs[0]), int(gs[-1])
        # windows of 128 graphs
        w0, w1 = g0 // P, g1 // P
        for w in range(w0, w1 + 1):
            glo = max(g0, w * P)
            ghi = min(g1, (w + 1) * P - 1)
            ncol = ghi - glo + 1
            Pt = np.zeros((P, ncol), np.float32)
            for p in range(hi - lo):
                g = int(gs[p])
                if w * P <= g <= ghi and g >= glo:
                    Pt[p, g - glo] = 1.0 / max(cnt_g[g], 1.0)
            pool_meta.append(dict(tile=t, window=w, grow=glo - w * P, ncol=ncol,
                                  col_off=sum(c.shape[1] for c in pool_cols)))
            pool_cols.append(Pt)
    pool_data = np.concatenate(pool_cols, axis=1) if pool_cols else np.zeros((P, 1), np.float32)

    return dict(
        N=N, E=E, I=I, G=G, RE=RE, EH=EH, NQ=NQ, nblk=nblk,
        deg_e=deg_e, deg_n=deg_n, Binv=Binv, Dinv=Dinv,
        core_of_edge=core_of_edge, epos=epos,
        sched1=sched1, sched2=sched2, schedp=schedp,
        n_p1_regions=len(p1_regions), p1_region_size=p1_region_size,
        n_p2_regions=len(p2_regions), p2_region_size=p2_region_size,
        pool_meta=pool_meta, pool_data=pool_data, cnt_g=cnt_g,
    )


def build_agg_fast(keys, percore_arrs, src_off_of, region_of, region_rows):
    """Vectorized build_agg. keys: sorted list of key tuples. percore_arrs:
    key -> list over cores of (dst[nseg], lens[nseg], src[np_], scale[np_])
    with segments in canonical (erow) order; pairs contiguous per segment.
    Matches build_agg semantics: per-core stable sort by -len, 128-seg
    windows, D = max first-len per window, chunks of sum(D)<=MAXCOLS."""
    s = AggSched()
    for key in keys:
        arrs = percore_arrs[key]
        nseg = max(len(a[0]) for a in arrs)
        if nseg == 0:
            continue
        src_off = src_off_of(key)
        region = region_of(key)
        rr = region_rows[region]
        # per-core sorted segment views
        sorted_data = []
        padlens = np.zeros((NCORES, nseg), np.int64)
        for c in range(NCORES):
            dst, lens, src, scale = arrs[c]
            nc_ = len(dst)
            so = np.argsort(-lens, kind="stable")
            lens_s = lens[so]
            dst_s = dst[so]
            ptr = np.concatenate([[0], np.cumsum(lens)])
            cnts = lens_s
            out_ptr = np.concatenate([[0], np.cumsum(cnts)])
            tot = int(out_ptr[-1])
            if tot:
                rep_seg = np.repeat(np.arange(nc_), cnts)
                krank = np.arange(tot) - np.repeat(out_ptr[:-1], cnts)
                pos = np.repeat(ptr[so], cnts) + krank
                src_s = src[pos]
                scale_s = scale[pos]
            else:
                rep_seg = np.zeros(0, np.int64)
                krank = np.zeros(0, np.int64)
                src_s = np.zeros(0, np.int64)
                scale_s = np.zeros(0, np.float32)
            padlens[c, :nc_] = lens_s
            sorted_data.append((dst_s, lens_s, out_ptr, rep_seg, krank, src_s, scale_s))
        nwin = (nseg + P - 1) // P
        # D per window = max over cores of first (largest) seg len in window
        Dw = np.maximum(padlens[:, ::P].max(axis=0), 1)
        # chunk grouping
        groups = []
        i = 0
        while i < nwin:
            j = i
            tot = 0
            while j < nwin and tot + Dw[j] <= MAXCOLS:
                tot += Dw[j]
                j += 1
            groups.append((i, j))
            i = j
        for (w0, w1) in groups:
            nw = w1 - w0
            cols = int(Dw[w0:w1].sum())
            colbase = np.zeros(nwin, np.int64)
            colbase[w0:w1] = np.concatenate([[0], np.cumsum(Dw[w0:w1])[:-1]])
            runs = []
            col0 = 0
            for w in range(w0, w1):
                D = int(Dw[w])
                if runs and runs[-1][2] == D and runs[-1][0] + runs[-1][1] * runs[-1][2] == col0:
                    runs[-1] = (runs[-1][0], runs[-1][1] + 1, D)
                else:
                    runs.append((col0, 1, D))
                col0 += D
            seg_lo, seg_hi = w0 * P, min(w1 * P, nseg)
            for c in range(NCORES):
                dst_s, lens_s, out_ptr, rep_seg, krank, src_s, scale_s = sorted_data[c]
                nc_ = len(dst_s)
                idx_block = np.zeros(cols * P, np.int64)
                sc_block = np.zeros((P, cols), np.float32)
                sidx_block = np.full(nw * P, rr, np.int64)
                lo_c, hi_c = min(seg_lo, nc_), min(seg_hi, nc_)
                if hi_c > lo_c:
                    segr = np.arange(lo_c, hi_c)
                    wi = segr // P - w0
                    pp = segr % P
                    sidx_block[wi * P + pp] = dst_s[lo_c:hi_c]
                    plo, phi = int(out_ptr[lo_c]), int(out_ptr[hi_c])
                    if phi > plo:
                        sseg = rep_seg[plo:phi]
                        kr = krank[plo:phi]
                        col = colbase[sseg // P] + kr
                        slot = col * P + (sseg % P)
                        idx_block[slot] = src_s[plo:phi] - src_off
                        sc_block[sseg % P, col] = scale_s[plo:phi]
                s.idx[c].append(_wrap16(idx_block))
                s.scale[c].append(sc_block)
                s.sidx[c].append(_wrap16(sidx_block))
            s.chunks.append(dict(
                src_off=src_off, n_slots=cols * P, cols=cols,
                idx_off=s.idx_cols, scale_off=s.scale_cols,
                runs=runs, region=region, n_rows=nw * P, nw=nw,
                sidx_off=s.sidx_cols))
            s.idx_cols += cols * P // 16
            s.scale_cols += cols
            s.sidx_cols += nw * P // 16
    return s


def _segment_arrays(corev, majorv, lvlv, erowv, srcv, scalev, maxc):
    """Group pairs into segments keyed (major, lvl) per core, segments in erow
    order, pairs contiguous. majorv encodes (region/block) ids. Returns
    (keys_sorted, dict key -> per-core arrays). lvlv must already be the
    within-(core,major,erow) rank // maxc."""
    n = len(corev)
    L = int(lvlv.max()) + 1 if n else 1
    EMAX = int(erowv.max()) + 2 if n else 1
    MMAX = int(majorv.max()) + 1 if n else 1
    keyv = (((corev * MMAX + majorv) * L + lvlv) * EMAX + erowv)
    o = np.argsort(keyv, kind="stable")
    ks = keyv[o]
    co, mo, lo_, eo = corev[o], majorv[o], lvlv[o], erowv[o]
    so, sco = srcv[o], scalev[o]
    newseg = np.empty(n, bool)
    if n:
        newseg[0] = True
        newseg[1:] = ks[1:] != ks[:-1]
    segid = np.cumsum(newseg) - 1 if n else np.zeros(0, np.int64)
    seg_start = np.nonzero(newseg)[0]
    seg_end = np.concatenate([seg_start[1:], [n]]) if n else np.zeros(0, np.int64)
    seg_core = co[seg_start]
    seg_major = mo[seg_start]
    seg_lvl = lo_[seg_start]
    seg_erow = eo[seg_start]
    seg_len = seg_end - seg_start
    keys = sorted(set(zip(seg_major.tolist(), seg_lvl.tolist())))
    out = {}
    for (m, lv) in keys:
        percore = []
        for c in range(NCORES):
            sel = np.nonzero((seg_core == c) & (seg_major == m) & (seg_lvl == lv))[0]
            if len(sel):
                dst = seg_erow[sel]
                lens = seg_len[sel]
                plo = seg_start[sel]
                cnts = lens
                pos = np.repeat(plo, cnts) + (np.arange(int(cnts.sum())) -
                                              np.repeat(np.concatenate([[0], np.cumsum(cnts)])[:-1], cnts))
                percore.append((dst, lens, so[pos], sco[pos]))
            else:
                percore.append((np.zeros(0, np.int64), np.zeros(0, np.int64),
                                np.zeros(0, np.int64), np.zeros(0, np.float32)))
        out[(m, lv)] = percore
    return keys, out


def _interleave_chunks(s):
    """Round-robin chunks across regions so consecutive scatters hit different
    tables (breaks Tile WAW chains; lets scatter DMAs overlap across queues).
    Stream blocks are permuted along with the chunks so the streams stay
    contiguous in emission order (enables coalesced block loads)."""
    by_reg = {}
    for i, ch in enumerate(s.chunks):
        by_reg.setdefault(ch["region"], []).append(i)
    lists = list(by_reg.values())
    order = []
    while any(lists):
        for L in lists:
            if L:
                order.append(L.pop(0))
    s.chunks = [s.chunks[i] for i in order]
    for c in range(NCORES):
        s.idx[c] = [s.idx[c][i] for i in order]
        s.scale[c] = [s.scale[c][i] for i in order]
        s.sidx[c] = [s.sidx[c][i] for i in order]
    io = so = xo = 0
    for ch in s.chunks:
        ch["idx_off"] = io
        io += ch["n_slots"] // 16
        ch["scale_off"] = so
        so += ch["cols"]
        ch["sidx_off"] = xo
        xo += ch["n_rows"] // 16


def _split_big(segs, maxc):
    # Oversized segments are split into levels; each level becomes its OWN key so
    # duplicate destinations never share a scatter instruction (CCE RMW races).
    for key in list(segs):
        base = segs.pop(key)
        levels = {}
        for c in range(NCORES):
            for reg, dst, pairs in base[c]:
                for lv, j in enumerate(range(0, len(pairs), maxc)):
                    levels.setdefault(lv, [[] for _ in range(NCORES)])[c].append(
                        (reg, dst, pairs[j:j + maxc]))
        for lv, percore in levels.items():
            segs[key + (lv,)] = percore


# ----------------------------------------------------------------------------
# numpy executor for schedule validation (mirrors device semantics)
# ----------------------------------------------------------------------------

def numpy_agg(sched, src_tables, region_tables, core):
    """src_tables: full source table [rows, F] (np); region_tables: list of zeroed np arrays."""
    F = 64
    for ch in sched.chunks:
        cols = ch["cols"]
        idx = _unwrap(np.concatenate(sched.idx[core], axis=1), ch["idx_off"], ch["n_slots"])
        sc = np.concatenate(sched.scale[core], axis=1)[:, ch["scale_off"]:ch["scale_off"] + cols]
        g = src_tables[ch["src_off"] + idx.reshape(cols, P)]  # [cols, P, F] slot=(c*128+p)
        g = g.transpose(1, 0, 2) * sc[:, :, None]             # [P, cols, F]
        stage = np.zeros((P, ch["nw"], F), np.float32)
        wi = 0
        for (c0, nw, D) in ch["runs"]:
            for k in range(nw):
                stage[:, wi] = g[:, c0 + k * D:c0 + (k + 1) * D].sum(axis=1)
                wi += 1
        sidx = _unwrap(np.concatenate(sched.sidx[core], axis=1), ch["sidx_off"], ch["n_rows"])
        rt = region_tables[ch["region"]]
        for i in range(ch["n_rows"]):
            r = sidx[i]
            rt[r] += stage[i % P, i // P]


def _unwrap(stream, off, n):
    blk = stream[:16, off:off + n // 16]
    return blk.T.reshape(-1)[:n].astype(np.int64)


# ----------------------------------------------------------------------------
# Bass kernel builder
# ----------------------------------------------------------------------------

def build_nc(pre, AD, NLAYERS=3, HD=128):
    import concourse.bass as bass
    import concourse.mybir as mybir
    from concourse import bacc
    from concourse.tile import TileContext
    from concourse.masks import make_identity

    F = 64
    AF = mybir.ActivationFunctionType
    OP = mybir.AluOpType
    f32 = mybir.dt.float32
    i16 = mybir.dt.int16

    N = pre["N"]; G = pre["G"]
    NPAD = ((N + 1023) // 1024) * 1024
    NT = NPAD // P                      # node tiles
    NCH = NT // 2                       # apply chunks (2 tiles each)
    EH = pre["EH"]; NQ = pre["NQ"]
    EF_ROWS = ((2 * EH + 127) // 128) * 128
    R1 = ((pre["p1_region_size"] + 127) // 128) * 128
    R2 = ((pre["p2_region_size"] + 127) // 128) * 128
    NR1 = pre["n_p1_regions"]; NR2 = pre["n_p2_regions"]
    s1, s2 = pre["sched1"], pre["sched2"]
    GW = (G + P - 1) // P               # pooled windows

    NS = NPAD // NCORES                 # per-core node slice rows
    NTS = NS // P                       # slice tiles (98)
    nc = bacc.Bacc(num_devices=NCORES, num_swdge_queues=4)
    # ---- inputs
    xT = nc.dram_tensor("xT", [AD, NS], f32, kind="ExternalInput")
    W_emb = nc.dram_tensor("W_emb", [AD, F], f32, kind="ExternalInput")
    b_emb = nc.dram_tensor("b_emb", [1, F], f32, kind="ExternalInput")
    convW2 = nc.dram_tensor("convW2", [NLAYERS * P, F], f32, kind="ExternalInput")
    conv_bc = nc.dram_tensor("conv_bc", [F, NLAYERS], f32, kind="ExternalInput")
    bn_gc = nc.dram_tensor("bn_gc", [F, NLAYERS], f32, kind="ExternalInput")
    bn_bc = nc.dram_tensor("bn_bc", [F, NLAYERS], f32, kind="ExternalInput")
    fc_W = nc.dram_tensor("fc_W", [F, HD], f32, kind="ExternalInput")
    fc_b = nc.dram_tensor("fc_b", [1, HD], f32, kind="ExternalInput")
    fco_W = nc.dram_tensor("fco_W", [HD, 1], f32, kind="ExternalInput")
    fco_b = nc.dram_tensor("fco_b", [1, 1], f32, kind="ExternalInput")
    p1_idx = nc.dram_tensor("p1_idx", [P, max(s1.idx_cols, 1)], i16, kind="ExternalInput")
    p1_sc = nc.dram_tensor("p1_sc", [P, max(s1.scale_cols, 1)], f32, kind="ExternalInput")
    p1_sx = nc.dram_tensor("p1_sx", [P, max(s1.sidx_cols, 1)], i16, kind="ExternalInput")
    p2_idx = nc.dram_tensor("p2_idx", [P, max(s2.idx_cols, 1)], i16, kind="ExternalInput")
    p2_sc = nc.dram_tensor("p2_sc", [P, max(s2.scale_cols, 1)], f32, kind="ExternalInput")
    p2_sx = nc.dram_tensor("p2_sx", [P, max(s2.sidx_cols, 1)], i16, kind="ExternalInput")
    sp = pre["schedp"]
    pl_idx = nc.dram_tensor("pl_idx", [P, max(sp.idx_cols, 1)], i16, kind="ExternalInput")
    pl_sc = nc.dram_tensor("pl_sc", [P, max(sp.scale_cols, 1)], f32, kind="ExternalInput")
    pl_sx = nc.dram_tensor("pl_sx", [P, max(sp.sidx_cols, 1)], i16, kind="ExternalInput")
    # ---- internal tables
    HTAB = nc.dram_tensor("HTAB", [NPAD, F], f32, addr_space="Shared")
    HSL = nc.dram_tensor("HSL", [NS, F], f32)
    RSO = nc.dram_tensor("RSO", [NS, F], f32)
    CSD = nc.dram_tensor("CSD", [F + 1, F], f32)
    CSG = nc.dram_tensor("CSG", [F + 1, F], f32, addr_space="Shared")
    EHP = ((EH + 127) // 128) * 128
    EFH = [nc.dram_tensor(f"EF{h}", [EHP, F], f32) for h in range(2)]
    REG1 = [nc.dram_tensor(f"REG1_{r}", [R1, F], f32) for r in range(NR1)]
    REG2 = [nc.dram_tensor(f"REG2_{r}", [R2, F], f32) for r in range(NR2)]
    ARIN = nc.dram_tensor("ARIN", [NPAD, F], f32)
    PGR = ((G + 1 + 127) // 128) * 128          # pooled partials rows (pad + dump)
    PGT = nc.dram_tensor("PGT", [PGR, F], f32)
    PGG = nc.dram_tensor("PGG", [PGR, F], f32, addr_space="Shared")
    OUT = nc.dram_tensor("OUT", [GW * P, 1], f32, kind="ExternalOutput")

    KB = int(os.environ.get("KBISECT", "9"))
    with TileContext(nc) as tc:
        with (
            tc.tile_pool(name="const", bufs=1) as cpool,
            tc.tile_pool(name="gbuf", bufs=3) as gpool,
            tc.tile_pool(name="stage", bufs=3) as spool,
            tc.tile_pool(name="small", bufs=2) as mpool,
            tc.tile_pool(name="dense", bufs=3) as dpool,
            tc.tile_pool(name="gbatch", bufs=13) as gbpool,
            tc.tile_pool(name="sbatch", bufs=13) as sbpool,
            tc.tile_pool(name="psum", bufs=4, space="PSUM") as ppool,
            tc.tile_pool(name="psA", bufs=1, space="PSUM") as psA,
        ):
            ident = cpool.tile([P, P], f32)
            make_identity(nc, ident[:])
            ones_col = cpool.tile([P, 1], f32)
            nc.gpsimd.memset(ones_col[:], 1.0)
            ones_row = cpool.tile([1, P], f32)
            nc.gpsimd.memset(ones_row[:], 1.0)
            zt = cpool.tile([P, 32, F], f32)
            nc.gpsimd.memset(zt[:], 0.0)

            # ---------- zero all regions + ARIN pad rows
            def zero_table(tab, rows):
                r3 = tab[:, :].rearrange("(k p) f -> p k f", p=P)
                K = rows // P
                for k0 in range(0, K, 32):
                    kk = min(32, K - k0)
                    nc.sync.dma_start(out=r3[:, k0:k0 + kk, :], in_=zt[:, :kk, :])
            for t in REG1:
                zero_table(t, R1)
            for t in REG2:
                zero_table(t, R2)
            zero_table(ARIN, NPAD)
            zero_table(PGT, PGR)

            # ---------- embedding: HTAB = x @ W_emb + b_emb
            wemb_t = cpool.tile([AD, F], f32)
            nc.sync.dma_start(out=wemb_t[:], in_=W_emb[:, :])
            bemb_t = cpool.tile([1, F], f32)
            nc.sync.dma_start(out=bemb_t[:], in_=b_emb[:, :])
            # broadcast b_emb to [128, 256] via PE: ones_row^T @ bemb
            bps = ppool.tile([P, F], f32, space="PSUM", tag="ps")
            nc.tensor.matmul(out=bps[:], lhsT=ones_row[:], rhs=bemb_t[:],
                             start=True, stop=True)
            bemb4 = cpool.tile([P, 4, F], f32)
            for j in range(4):
                nc.vector.tensor_copy(out=bemb4[:, j, :], in_=bps[:])
            h3 = HTAB[:, :].rearrange("(t p) f -> p t f", p=P)
            hs3 = HSL[:, :].rearrange("(t p) f -> p t f", p=P)
            for t0 in range(0, NTS, 4):
                tt = min(4, NTS - t0)
                eps_ = ppool.tile([P, 4 * F], f32, space="PSUM", tag="ps")
                for j in range(tt):
                    xc = gpool.tile([AD, P], f32)
                    nc.sync.dma_start(out=xc[:], in_=xT[:, (t0 + j) * P:(t0 + j + 1) * P])
                    nc.tensor.matmul(out=eps_[:, j * F:(j + 1) * F], lhsT=xc[:],
                                     rhs=wemb_t[:], start=True, stop=True)
                hb = spool.tile([P, 4, F], f32)
                nc.vector.tensor_tensor(out=hb[:, :tt, :],
                                        in0=eps_[:, :tt * F].rearrange("p (t f) -> p t f", f=F),
                                        in1=bemb4[:, :tt, :], op=OP.add)
                nc.sync.dma_start(out=hs3[:, t0:t0 + tt, :], in_=hb[:, :tt, :])
            nc.gpsimd.collective_compute(
                "AllGather", OP.bypass, replica_groups=[list(range(NCORES))],
                ins=[HSL[:, :]], outs=[HTAB[:, :]])

            # ---------- layers
            ar3 = ARIN[:, :].rearrange("(t p) f -> p t f", p=P)
            for l in range(NLAYERS if KB >= 8 else (1 if KB >= 1 else 0)):
                # ===== P1: gather HTAB -> piece regions
                def agg_pass(sched, idx_d, sc_d, sx_d, src3, src_rows, regs, use_scale,
                             src_tabs=None, src_div=1):
                    chs = sched.chunks
                    BCH = 16  # stream chunks per coalesced load
                    pend = []   # staged scatters from the previous sub-batch
                    for b0 in range(0, len(chs), BCH):
                        grp = chs[b0:b0 + BCH]
                        ni16 = sum(ch["n_slots"] for ch in grp) // 16
                        nsc = sum(ch["cols"] for ch in grp)
                        nsx = sum(ch["n_rows"] for ch in grp) // 16
                        it = gpool.tile([P, BCH * MAXNI // 16], i16, tag="itb")
                        nc.sync.dma_start(
                            out=it[:, :ni16],
                            in_=idx_d[:, grp[0]["idx_off"]:grp[0]["idx_off"] + ni16])
                        stb = gpool.tile([P, BCH * MAXCOLS], f32, tag="stb")
                        nc.sync.dma_start(
                            out=stb[:, :nsc],
                            in_=sc_d[:, grp[0]["scale_off"]:grp[0]["scale_off"] + nsc])
                        xtb = gpool.tile([P, BCH * MAXNI // 16], i16, tag="xtb")
                        nc.sync.dma_start(
                            out=xtb[:, :nsx],
                            in_=sx_d[:, grp[0]["sidx_off"]:grp[0]["sidx_off"] + nsx])
                        GB = 6   # gathers issued back-to-back before their scatters
                        for g0 in range(0, len(grp), GB):
                            sub = grp[g0:g0 + GB]
                            gathered = []
                            for k, ch in enumerate(sub):
                                ci = b0 + g0 + k
                                q = ci % 4
                                cols = ch["cols"]
                                io_ = ch["idx_off"] - grp[0]["idx_off"]
                                g = gbpool.tile([P, MAXCOLS, F], f32, tag="g")
                                if src_tabs is not None:
                                    tab = src_tabs[ch["src_off"] // src_div]
                                    in_ap_ = tab[0:min(32760, src_rows), :]
                                else:
                                    hi_ = min(ch["src_off"] + 32760, src_rows)
                                    in_ap_ = src3[ch["src_off"]:hi_, :]
                                nc.gpsimd.dma_gather(
                                    out_ap=g[:, :cols, :], in_ap=in_ap_,
                                    idxs_ap=it[:, io_:io_ + ch["n_slots"] // 16],
                                    num_idxs=ch["n_slots"], num_idxs_reg=ch["n_slots"],
                                    elem_size=F, queue_num=q)
                                gathered.append((g, ch, q))
                            for (stg_, ch_, q_, xtb_, xo2_) in pend:
                                nc.gpsimd.dma_scatter_add(
                                    out_ap=regs[ch_["region"]][:, :],
                                    in_ap=stg_[:, :ch_["nw"], :],
                                    idxs_ap=xtb_[:, xo2_:xo2_ + ch_["n_rows"] // 16],
                                    num_idxs=ch_["n_rows"], num_idxs_reg=ch_["n_rows"],
                                    elem_size=F, queue_num=q_)
                            staged = []
                            for (g, ch, q) in gathered:
                                cols = ch["cols"]
                                so_ = ch["scale_off"] - grp[0]["scale_off"]
                                if use_scale:
                                    gs = g
                                    nc.vector.tensor_tensor(
                                        out=gs[:, :cols, :], in0=g[:, :cols, :],
                                        in1=stb[:, so_:so_ + cols].to_broadcast([P, cols, F]),
                                        op=OP.mult)
                                else:
                                    gs = g
                                stg = sbpool.tile([P, MAXCOLS, F], f32, tag="stg")
                                wi = 0
                                for (c0, rnw, D) in ch["runs"]:
                                    if D == 1:
                                        nc.vector.tensor_copy(out=stg[:, wi:wi + rnw, :],
                                                              in_=gs[:, c0:c0 + rnw, :])
                                    else:
                                        src = gs[:, c0:c0 + rnw * D, :].rearrange(
                                            "p (w d) f -> p w f d", w=rnw, d=D)
                                        nc.vector.tensor_reduce(
                                            out=stg[:, wi:wi + rnw, :], in_=src,
                                            axis=mybir.AxisListType.X, op=OP.add)
                                    wi += rnw
                                staged.append(
                                    (stg, ch, q, xtb,
                                     ch["sidx_off"] - grp[0]["sidx_off"]))
                            pend = staged
                    for (stg_, ch_, q_, xtb_, xo2_) in pend:
                        nc.gpsimd.dma_scatter_add(
                            out_ap=regs[ch_["region"]][:, :],
                            in_ap=stg_[:, :ch_["nw"], :],
                            idxs_ap=xtb_[:, xo2_:xo2_ + ch_["n_rows"] // 16],
                            num_idxs=ch_["n_rows"], num_idxs_reg=ch_["n_rows"],
                            elem_size=F, queue_num=q_)

                agg_pass(s1, p1_idx, p1_sc, p1_sx, HTAB, NPAD, REG1, True)
                if KB == 1: continue

                # ===== L2-P1: EF[h*EH + r] = sum_b REG1[(b,h)][r]
                nblk = pre["nblk"]
                for h in range(2):
                    rh = [REG1[b * 2 + h] for b in range(nblk)]
                    KH = (EH + 127) // 128  # tiles of 128 rows (region rows >= EH)
                    for k0 in range(0, KH, 8):
                        kk = min(8, KH - k0)
                        mt = spool.tile([P, 4, 8, F], f32, tag="mt")
                        if nblk < 4:
                            nc.gpsimd.memset(mt[:], 0.0)
                        for b in range(nblk):
                            rb3 = rh[b][:, :].rearrange("(t p) f -> p t f", p=P)
                            nc.sync.dma_start(out=mt[:, b, :kk, :], in_=rb3[:, k0:k0 + kk, :])
                        red = spool.tile([P, 8, F], f32, tag="red")
                        src = mt[:, :, :kk, :].rearrange("p b w f -> p w f b")
                        nc.vector.tensor_reduce(out=red[:, :kk, :], in_=src,
                                                axis=mybir.AxisListType.X, op=OP.add)
                        off = k0 * P
                        nc.sync.dma_start(
                            out=EFH[h][off:off + kk * P, :].rearrange("(t p) f -> p t f", p=P),
                            in_=red[:, :kk, :])

                if l + 1 < (NLAYERS if KB >= 8 else 1):
                    for t_ in REG1:   # re-zero for next layer; overlaps P2/dense
                        zero_table(t_, R1)
                if KB == 2: continue
                # ===== P2: gather EF -> partial piece regions (scale = Binv*Dinv)
                agg_pass(s2, p2_idx, p2_sc, p2_sx, None, EHP, REG2, True,
                         src_tabs=EFH, src_div=EH)

                if KB == 3: continue
                # ===== L2-P2: ARIN[n] = sum_h REG2[(h, q)][n - q*NQ]
                for q in range(4):
                    lo = q * NQ
                    hi = min(lo + NQ, N)
                    rows = hi - lo
                    KH = (rows + 127) // 128
                    for k0 in range(0, KH, 8):
                        kk = min(8, KH - k0)
                        mt = spool.tile([P, 2, 8, F], f32, tag="mt")
                        for h in range(2):
                            rb3 = REG2[h * 4 + q][:, :].rearrange(
                                "(t p) f -> p t f", p=P)
                            nc.sync.dma_start(out=mt[:, h, :kk, :], in_=rb3[:, k0:k0 + kk, :])
                        red = spool.tile([P, 8, F], f32, tag="red")
                        src = mt[:, :, :kk, :].rearrange("p b w f -> p w f b")
                        nc.vector.tensor_reduce(out=red[:, :kk, :], in_=src,
                                                axis=mybir.AxisListType.X, op=OP.add)
                        # ARIN rows lo + k0*128 ... may exceed hi on last block; host
                        # guarantees NQ % 128 == 0 except last quarter; clamp rows:
                        wlim = min(kk * P, rows - k0 * P)
                        full_w = wlim // P
                        if full_w > 0:
                            nc.sync.dma_start(
                                out=ARIN[lo + k0 * P: lo + k0 * P + full_w * P, :]
                                    .rearrange("(t p) f -> p t f", p=P),
                                in_=red[:, :full_w, :])
                        remp = wlim - full_w * P
                        if remp > 0:
                            nc.sync.dma_start(
                                out=ARIN[lo + (k0 + full_w) * P: lo + (k0 + full_w) * P + remp, :],
                                in_=red[:remp, full_w, :])

                if l + 1 < (NLAYERS if KB >= 8 else 1):
                    for t_ in REG2:   # re-zero for next layer; overlaps dense/AG
                        zero_table(t_, R2)
                if KB == 4: continue
                is_last = (l == (NLAYERS - 1 if KB >= 8 else 0))
                # ===== cross-core reduce: RS -> per-core slice
                nc.gpsimd.collective_compute(
                    "ReduceScatter", OP.add, replica_groups=[list(range(NCORES))],
                    ins=[ARIN[:, :]],
                    outs=[RSO[:, :]])

                if KB == 5: continue
                # KB>=6: dense runs
                # ===== stats on slice: C = raw2^T raw2, S1 = raw2^T 1
                o3s = RSO[:, :].rearrange("(t p) f -> p t f", p=P)
                Cps = psA.tile([F, F], f32, space="PSUM", tag="C")
                Sps = psA.tile([F, 1], f32, space="PSUM", tag="S")
                Cs = mpool.tile([F, F], f32, tag="Cs")
                Ss = mpool.tile([F, 1], f32, tag="Ss")
                for t0 in range(0, NTS, 8):
                    tt = min(8, NTS - t0)
                    rt8 = dpool.tile([P, 8, F], f32, tag="rt8")
                    nc.sync.dma_start(out=rt8[:, :tt, :], in_=o3s[:, t0:t0 + tt, :])
                    for j in range(tt):
                        t = t0 + j
                        nc.tensor.matmul(out=Cps[:], lhsT=rt8[:, j, :], rhs=rt8[:, j, :],
                                         start=(t == 0), stop=(t == NTS - 1))
                        nc.tensor.matmul(out=Sps[:], lhsT=rt8[:, j, :], rhs=ones_col[:],
                                         start=(t == 0), stop=(t == NTS - 1))
                # pack [C ; S^T] -> tiny AllReduce -> global stats
                Csl = mpool.tile([F, F], f32, tag="Csl")
                nc.vector.tensor_copy(out=Csl[:], in_=Cps[:])
                Ssl = mpool.tile([F, 1], f32, tag="Ssl")
                nc.vector.tensor_copy(out=Ssl[:], in_=Sps[:])
                srps = ppool.tile([1, F], f32, space="PSUM", tag="ps")
                nc.tensor.matmul(out=srps[:], lhsT=Ssl[:], rhs=ident[:F, :F],
                                 start=True, stop=True)
                srow = mpool.tile([1, F], f32, tag="srow")
                nc.vector.tensor_copy(out=srow[:], in_=srps[:])
                nc.sync.dma_start(out=CSD[0:F, :], in_=Csl[:])
                nc.sync.dma_start(out=CSD[F:F + 1, :], in_=srow[:])
                nc.gpsimd.collective_compute(
                    "AllReduce", OP.add, replica_groups=[list(range(NCORES))],
                    ins=[CSD[:, :]], outs=[CSG[:, :]])
                nc.sync.dma_start(out=Cs[:], in_=CSG[0:F, :])
                sgrow = mpool.tile([1, F], f32, tag="sgrow")
                nc.sync.dma_start(out=sgrow[:], in_=CSG[F:F + 1, :])
                sgps = ppool.tile([F, 1], f32, space="PSUM", tag="ps")
                nc.tensor.matmul(out=sgps[:], lhsT=sgrow[:], rhs=ones_col[:1, :],
                                 start=True, stop=True)
                nc.vector.tensor_copy(out=Ss[:], in_=sgps[:])

                if KB == 60: continue
                # ===== postprocess: a, cfin, Wp2, cb2
                Wl = cpool.tile([P, F], f32, tag="Wl")
                nc.sync.dma_start(out=Wl[:], in_=convW2[l * P:(l + 1) * P, :])
                bcol = mpool.tile([F, 1], f32)
                nc.sync.dma_start(out=bcol[:], in_=conv_bc[:, l:l + 1])
                gcol = mpool.tile([F, 1], f32)
                nc.sync.dma_start(out=gcol[:], in_=bn_gc[:, l:l + 1])
                btcol = mpool.tile([F, 1], f32)
                nc.sync.dma_start(out=btcol[:], in_=bn_bc[:, l:l + 1])
                mps = ppool.tile([F, 1], f32, space="PSUM", tag="ps")
                nc.tensor.matmul(out=mps[:], lhsT=Wl[:F, :], rhs=Ss[:], start=True, stop=True)
                mrw = mpool.tile([F, 1], f32)
                nc.scalar.activation(out=mrw[:], in_=mps[:], func=AF.Copy, scale=1.0 / N)
                t1ps = ppool.tile([F, F], f32, space="PSUM", tag="ps")
                nc.tensor.matmul(out=t1ps[:], lhsT=Cs[:], rhs=Wl[:F, :], start=True, stop=True)
                wt1 = mpool.tile([F, F], f32)
                nc.vector.tensor_tensor(out=wt1[:], in0=t1ps[:], in1=Wl[:F, :], op=OP.mult)
                e2ps = ppool.tile([F, 1], f32, space="PSUM", tag="ps")
                nc.tensor.matmul(out=e2ps[:], lhsT=wt1[:], rhs=ones_col[:F, :], start=True, stop=True)
                var = mpool.tile([F, 1], f32)
                nc.scalar.activation(out=var[:], in_=e2ps[:], func=AF.Copy, scale=1.0 / N)
                msq = mpool.tile([F, 1], f32)
                nc.vector.tensor_tensor(out=msq[:], in0=mrw[:], in1=mrw[:], op=OP.mult)
                nc.vector.tensor_tensor(out=var[:], in0=var[:], in1=msq[:], op=OP.subtract)
                nc.vector.tensor_scalar_add(out=var[:], in0=var[:], scalar1=1e-5)
                lnv = mpool.tile([F, 1], f32)
                nc.scalar.activation(out=lnv[:], in_=var[:], func=AF.Ln)
                rstd = mpool.tile([F, 1], f32)
                nc.scalar.activation(out=rstd[:], in_=lnv[:], func=AF.Exp, scale=-0.5)
                a_ = mpool.tile([F, 1], f32)
                nc.vector.tensor_tensor(out=a_[:], in0=gcol[:], in1=rstd[:], op=OP.mult)
                am = mpool.tile([F, 1], f32)
                nc.vector.tensor_tensor(out=am[:], in0=a_[:], in1=mrw[:], op=OP.mult)
                cfin = mpool.tile([F, 1], f32)
                nc.vector.tensor_tensor(out=cfin[:], in0=btcol[:], in1=am[:], op=OP.subtract)
                # rows
                arps = ppool.tile([1, F], f32, space="PSUM", tag="ps")
                nc.tensor.matmul(out=arps[:], lhsT=a_[:], rhs=ident[:F, :F], start=True, stop=True)
                arow = mpool.tile([1, F], f32)
                nc.vector.tensor_copy(out=arow[:], in_=arps[:])
                crps = ppool.tile([1, F], f32, space="PSUM", tag="ps")
                nc.tensor.matmul(out=crps[:], lhsT=cfin[:], rhs=ident[:F, :F], start=True, stop=True)
                crow = mpool.tile([1, F], f32)
                nc.vector.tensor_copy(out=crow[:], in_=crps[:])
                abps = ppool.tile([P, F], f32, space="PSUM", tag="ps")
                nc.tensor.matmul(out=abps[:], lhsT=ones_row[:], rhs=arow[:], start=True, stop=True)
                Wp2 = cpool.tile([P, F], f32, tag="Wp")
                nc.vector.tensor_tensor(out=Wp2[:], in0=Wl[:], in1=abps[:], op=OP.mult)
                cbps = ppool.tile([P, F], f32, space="PSUM", tag="ps")
                nc.tensor.matmul(out=cbps[:], lhsT=ones_row[:], rhs=crow[:], start=True, stop=True)
                cb4 = cpool.tile([P, 4, F], f32, tag="cb")
                for jj in range(4):
                    nc.vector.tensor_copy(out=cb4[:, jj, :], in_=cbps[:])

                if KB == 61: continue
                # ===== apply on the RS slice: h = softplus(raw2 @ Wp2 + cb4)
                for t0a in range(0, NTS, 4):
                    ta = min(4, NTS - t0a)
                    rc = dpool.tile([P, 4, F], f32)
                    nc.sync.dma_start(out=rc[:, :ta, :], in_=o3s[:, t0a:t0a + ta, :])
                    yps = ppool.tile([P, 4 * F], f32, space="PSUM", tag="ps")
                    for j in range(ta):
                        trp = ppool.tile([F, P], f32, space="PSUM", tag="ps")
                        nc.tensor.transpose(out=trp[:], in_=rc[:, j, :], identity=ident[:])
                        trs = dpool.tile([F, P], f32, tag="trs")
                        nc.vector.tensor_copy(out=trs[:], in_=trp[:])
                        nc.tensor.matmul(out=yps[:, j * F:(j + 1) * F], lhsT=trs[:],
                                         rhs=Wp2[0:F, :], start=True, stop=True)
                    yb = dpool.tile([P, 4 * F], f32)
                    nc.vector.tensor_tensor(out=yb[:, :ta * F],
                                            in0=yps[:, :ta * F],
                                            in1=cb4[:, :ta, :].rearrange("p t f -> p (t f)"),
                                            op=OP.add)
                    ex = dpool.tile([P, 4 * F], f32)
                    nc.scalar.activation(out=ex[:, :ta * F], in_=yb[:, :ta * F], func=AF.Exp)
                    hc = dpool.tile([P, 4, F], f32)
                    nc.scalar.activation(out=hc[:, :ta, :].rearrange("p t f -> p (t f)"),
                                         in_=ex[:, :ta * F], func=AF.Ln, bias=1.0, scale=1.0)
                    nc.sync.dma_start(out=hs3[:, t0a:t0a + ta, :], in_=hc[:, :ta, :])
                if not is_last:
                    nc.gpsimd.collective_compute(
                        "AllGather", OP.bypass, replica_groups=[list(range(NCORES))],
                        ins=[HSL[:, :]], outs=[HTAB[:, :]])
                elif KB not in (60, 61, 62):
                    # ===== pooling: slice segment-sum by graph (agg machinery on
                    # HSL), then AllReduce the per-graph partials
                    agg_pass(sp, pl_idx, pl_sc, pl_sx, HSL, NS, [PGT], True)
                    nc.gpsimd.collective_compute(
                        "AllReduce", OP.add, replica_groups=[list(range(NCORES))],
                        ins=[PGT[:, :]], outs=[PGG[:, :]])

            # ---------- FC head
            if KB < 6 or KB in (60, 61, 62):
                dz = dpool.tile([P, 1], f32)
                nc.gpsimd.memset(dz[:], 0.0)
                for w in range(GW):
                    nc.sync.dma_start(out=OUT[w * P:(w + 1) * P, :], in_=dz[:])
            fcw_t = cpool.tile([F, HD], f32)
            nc.sync.dma_start(out=fcw_t[:], in_=fc_W[:, :])
            fcb_t = mpool.tile([1, HD], f32)
            nc.sync.dma_start(out=fcb_t[:], in_=fc_b[:, :])
            fcbps = ppool.tile([P, HD], f32, space="PSUM", tag="ps")
            nc.tensor.matmul(out=fcbps[:], lhsT=ones_row[:], rhs=fcb_t[:], start=True, stop=True)
            fcb_b = cpool.tile([P, HD], f32)
            nc.vector.tensor_copy(out=fcb_b[:], in_=fcbps[:])
            fcow_t = cpool.tile([HD, 1], f32)
            nc.sync.dma_start(out=fcow_t[:], in_=fco_W[:, :])
            fcob_t = mpool.tile([1, 1], f32)
            nc.sync.dma_start(out=fcob_t[:], in_=fco_b[:, :])
            fcobps = ppool.tile([P, 1], f32, space="PSUM", tag="ps")
            nc.tensor.matmul(out=fcobps[:], lhsT=ones_row[:], rhs=fcob_t[:], start=True, stop=True)
            fcob_b = cpool.tile([P, 1], f32)
            nc.vector.tensor_copy(out=fcob_b[:], in_=fcobps[:])
            for w in range(GW if (KB >= 6 and KB not in (60, 61, 62)) else 0):
                pgt_ = dpool.tile([P, F], f32)
                nc.sync.dma_start(out=pgt_[:], in_=PGG[w * P:(w + 1) * P, :])
                pgps = ppool.tile([F, P], f32, space="PSUM", tag="ps")
                nc.tensor.transpose(out=pgps[:], in_=pgt_[:], identity=ident[:])
                pgs = dpool.tile([F, P], f32)
                nc.vector.tensor_copy(out=pgs[:], in_=pgps[:])
                gts = dpool.tile([F, P], f32)
                ge_ = dpool.tile([F, P], f32)
                nc.scalar.activation(out=ge_[:], in_=pgs[:], func=AF.Exp)
                nc.scalar.activation(out=gts[:], in_=ge_[:], func=AF.Ln, bias=1.0, scale=1.0)
                g2ps = ppool.tile([P, HD], f32, space="PSUM", tag="ps")
                nc.tensor.matmul(out=g2ps[:], lhsT=gts[:], rhs=fcw_t[:], start=True, stop=True)
                g2b = dpool.tile([P, HD], f32)
                nc.vector.tensor_tensor(out=g2b[:], in0=g2ps[:], in1=fcb_b[:], op=OP.add)
                g2e = dpool.tile([P, HD], f32)
                nc.scalar.activation(out=g2e[:], in_=g2b[:], func=AF.Exp)
                g2 = dpool.tile([P, HD], f32)
                nc.scalar.activation(out=g2[:], in_=g2e[:], func=AF.Ln, bias=1.0, scale=1.0)
                g2tp = ppool.tile([P, P], f32, space="PSUM", tag="ps")
                nc.tensor.transpose(out=g2tp[:], in_=g2[:], identity=ident[:])
                g2ts = dpool.tile([P, P], f32)
                nc.vector.tensor_copy(out=g2ts[:], in_=g2tp[:])
                y_ps = ppool.tile([P, 1], f32, space="PSUM", tag="ps")
                nc.tensor.matmul(out=y_ps[:], lhsT=g2ts[:], rhs=fcow_t[:], start=True, stop=True)
                y_t = dpool.tile([P, 1], f32)
                nc.vector.tensor_tensor(out=y_t[:], in0=y_ps[:], in1=fcob_b[:], op=OP.add)
                nc.sync.dma_start(out=OUT[w * P:(w + 1) * P, :], in_=y_t[:])
    return nc


def _streams(pre, core):
    s1, s2, sp = pre["sched1"], pre["sched2"], pre["schedp"]
    def cat(lst, dtype, n):
        if not lst:
            return np.zeros((P, 1), dtype)
        a = np.concatenate(lst, axis=1).astype(dtype)
        assert a.shape[1] == n, (a.shape, n)
        return a
    return dict(
        p1_idx=cat(s1.idx[core], np.int16, s1.idx_cols),
        p1_sc=cat(s1.scale[core], np.float32, s1.scale_cols),
        p1_sx=cat(s1.sidx[core], np.int16, s1.sidx_cols),
        p2_idx=cat(s2.idx[core], np.int16, s2.idx_cols),
        p2_sc=cat(s2.scale[core], np.float32, s2.scale_cols),
        p2_sx=cat(s2.sidx[core], np.int16, s2.sidx_cols),
        pl_idx=cat(sp.idx[core], np.int16, sp.idx_cols),
        pl_sc=cat(sp.scale[core], np.float32, sp.scale_cols),
        pl_sx=cat(sp.sidx[core], np.int16, sp.sidx_cols),
    )


def prepare(x, W_emb, b_emb, conv_W, conv_b, bn_gamma, bn_beta,
            fc_W, fc_b, fco_W, fco_b, node_idx, edge_idx, batch):
    """Build (nc, in_maps, G) — the finalized Bass module and per-core inputs."""
    x = np.asarray(x, np.float32)
    N, AD = x.shape
    G = int(np.asarray(batch).max()) + 1
    pre = host_prep(x, np.asarray(node_idx, np.int64), np.asarray(edge_idx, np.int64),
                    np.asarray(batch, np.int64))
    NL = np.asarray(conv_W).shape[0]
    nc = build_nc(pre, AD, NLAYERS=NL, HD=np.asarray(fc_W).shape[1])
    nc.finalize()
    NPAD = ((N + 1023) // 1024) * 1024
    NS = NPAD // NCORES
    xTp = np.zeros((AD, NPAD), np.float32)
    xTp[:, :N] = x.T
    convW2 = np.concatenate([np.concatenate([w, w], axis=0) for w in np.asarray(conv_W, np.float32)], axis=0)
    common = dict(
        W_emb=np.asarray(W_emb, np.float32),
        b_emb=np.asarray(b_emb, np.float32).reshape(1, -1),
        convW2=convW2,
        conv_bc=np.asarray(conv_b, np.float32).T.copy(),
        bn_gc=np.asarray(bn_gamma, np.float32).T.copy(),
        bn_bc=np.asarray(bn_beta, np.float32).T.copy(),
        fc_W=np.asarray(fc_W, np.float32),
        fc_b=np.asarray(fc_b, np.float32).reshape(1, -1),
        fco_W=np.asarray(fco_W, np.float32),
        fco_b=np.asarray(fco_b, np.float32).reshape(1, 1),
    )
    in_maps = []
    for c in range(NCORES):
        m = dict(common)
        m["xT"] = xTp[:, c * NS:(c + 1) * NS].copy()
        m.update(_streams(pre, c))
        in_maps.append(m)
    return nc, in_maps, G


def kernel(x, W_emb, b_emb, conv_W, conv_b, bn_gamma, bn_beta,
           fc_W, fc_b, fco_W, fco_b, node_idx, edge_idx, batch, use_sim=False):
    from concourse.bass_utils import run_bass_kernel_spmd
    nc, in_maps, G = prepare(x, W_emb, b_emb, conv_W, conv_b, bn_gamma, bn_beta,
                             fc_W, fc_b, fco_W, fco_b, node_idx, edge_idx, batch)
    if use_sim:
        from concourse.bass_interp import MultiCoreSim
        sim = MultiCoreSim(nc, num_cores=NCORES, num_workers=8)
        for cid, cs in sim.cores.items():
            for name, arr in in_maps[cid].items():
                cs.tensor(name)[:] = arr
        sim.simulate()
        out = np.array(sim.cores[0].tensor("OUT"))[:G]
        return out.astype(np.float32)
    res = run_bass_kernel_spmd(nc, in_maps, core_ids=list(range(NCORES)), trace=False)
    global LAST_EXEC_NS
    LAST_EXEC_NS = res.exec_time_ns
    out = res.results[0]["OUT"][:G]
    return out.astype(np.float32)

